# revision 55
# baseline (speedup 1.0000x reference)
"""AWD-LSTM forward on 8 Trainium2 NeuronCores (Bass/Tile, SPMD).

Strategy:
  - 8-way tensor parallelism over the hidden/gate dimension for all three LSTM
    layers (H=1150 -> padded 1280, 160 rows/core; layer2 H=400 -> 512, 64/core),
    with a per-step AllGather of h (bf16).
  - Everything lives in transposed layout: features on partitions, batch (80)
    on the free dim.  Per-step gates G.T[:, b] accumulate in PSUM from
    (a) Whh_slice.T.T @ h.T (recurrent) and, for layers 1/2, (b) Wih_slice @ x.T
    fused into the same accumulation group.  Layer0's input contribution is a
    bulk GEMM from the (host-gathered) embedding, staged through DRAM.
  - Vocab projection is sharded over V (33280/8 = 4160 cols/core), computed in
    time-chunks from the accumulated ys.T history, PSUM DMA'd straight to HBM.
  - Emission follows the wavefront (slot s: L0 step s, L1 step s-1, L2 step
    s-2, due xw0/decode chunks) so each engine's in-order stream interleaves
    layers and the AllGather latency is hidden by compute of other layers.
  - bf16 matmuls, fp32 cell state & PSUM accumulation.
"""
import os
import sys

sys.path.insert(0, '/opt/trn_rl_repo')

import numpy as np
import ml_dtypes

BF16 = ml_dtypes.bfloat16

# problem shapes (hardcoded per contract)
V, E, H, T, B = 33278, 400, 1150, 70, 80
NCORES = 8
EP = 512            # padded E (K dim)
HP = 1280           # padded H (layers 0,1)
SL = HP // NCORES   # 160 hidden rows per core
H2P = 512           # padded layer-2 H (=400)
SL2 = H2P // NCORES  # 64
VP = 33280
VS = VP // NCORES   # 4160
NKE = EP // 128     # 4   K-tiles over embedding dim
NKH = HP // 128     # 10  K-tiles over H (layers 0,1)
NKH2 = H2P // 128   # 4   K-tiles over layer-2 H
NM01 = 5            # M-tiles of per-core gates, layers 0/1 (640 rows)
NM2 = 2             # M-tiles of per-core gates, layer 2 (256 rows)
TCH = 8             # decode/xw0 time-chunk (steps)


# ----------------------------------------------------------------------------
# host-side prep
# ----------------------------------------------------------------------------

def _slice_rows_l01(k):
    """Core k's hidden rows (padded H=1280): a 128-block plus a 32-block.

    This split makes the combined-AllGather output (per-rank blocks of
    [h0(128+32) h1(128+32) h2(64)]) map back onto plain contiguous 128-row
    K-tiles with affine DMA access patterns.
    """
    return list(range(128 * k, 128 * (k + 1))) + \
        list(range(1024 + 32 * k, 1024 + 32 * (k + 1)))


def _gate_row_map_l01(k):
    # M order: [i_lo(128) f_lo(128) o_lo(128) g_lo(128) | i_hi f_hi o_hi g_hi (32 each)]
    # orig (PyTorch) gate order: i=0 f=1 g=2 o=3;  ours [i,f,o,g] -> [0,1,3,2]
    order = [0, 1, 3, 2]
    sr = _slice_rows_l01(k)
    rows = []
    for m in range(640):
        if m < 512:
            gate = order[m // 128]
            j = sr[m % 128]
        else:
            sub = m - 512
            gate = order[sub // 32]
            j = sr[128 + (sub % 32)]
        rows.append((gate, j))
    return rows


def _gate_row_map_l2(k):
    order = [0, 1, 3, 2]
    return [(order[m // 64], SL2 * k + (m % 64)) for m in range(256)]


def _make_wT(Wih, Whh, bih, bhh, k, hsz, row_map, in_sz, in_pad, hpad):
    nM = len(row_map)
    wi = np.zeros((in_pad, nM), np.float32)
    wh = np.zeros((hpad, nM), np.float32)
    b = np.zeros(nM, np.float32)
    for m, (gate, j) in enumerate(row_map):
        if j < hsz:
            r = gate * hsz + j
            wi[:in_sz, m] = Wih[r]
            wh[:hsz, m] = Whh[r]
            b[m] = bih[r] + bhh[r]
    return wi.astype(BF16), wh.astype(BF16), b


def _fold_k(a, nk):
    """[nk*128, N] -> [128, nk, N] with row r=(kt*128+p) -> [p, kt]."""
    n = a.shape[1]
    return np.ascontiguousarray(a.reshape(nk, 128, n).transpose(1, 0, 2))


def _bias_tiles_l01(b):
    # b: [640] (4x128 lo + 4x32 hi) -> [128, 5]: cols 0:4 lo gates, col 4 = hi combo
    out = np.zeros((128, 5), np.float32)
    for g in range(4):
        out[:, g] = b[g * 128:(g + 1) * 128]
        out[32 * g:32 * (g + 1), 4] = b[512 + g * 32: 512 + (g + 1) * 32]
    return out


def _bias_tiles_l2(b):
    # b: [256] (4x64, order i f o g) -> [128, 2]: col0 = [i|f], col1 = [o|g]
    out = np.zeros((128, 2), np.float32)
    out[:64, 0] = b[0:64]
    out[64:, 0] = b[64:128]
    out[:64, 1] = b[128:192]
    out[64:, 1] = b[192:256]
    return out


def _full_hT(h0, hsz, hpad, nk):
    out = np.zeros((hpad, B), np.float32)
    out[:hsz] = np.asarray(h0, np.float32).T
    return _fold_k(out.astype(BF16), nk)


def _c_pack_l01(c0, k):
    # per-core c slice -> [128, 2, 80] (col0 = 128-block, col1[:32] = 32-block)
    out = np.zeros((128, 2, B), np.float32)
    c0 = np.asarray(c0, np.float32)
    sr = _slice_rows_l01(k)
    sl = np.zeros((SL, B), np.float32)
    for i, j in enumerate(sr):
        if j < H:
            sl[i] = c0[:, j]
    out[:, 0, :] = sl[:128]
    out[:32, 1, :] = sl[128:160]
    return out


def _c_pack_l2(c0, k):
    out = np.zeros((128, B), np.float32)
    lo, hi = SL2 * k, min(E, SL2 * (k + 1))
    if hi > lo:
        out[:hi - lo] = np.asarray(c0, np.float32)[:, lo:hi].T
    return out


def prepare_inputs(inputs, t_steps=T):
    """Returns in_maps: list of 8 dicts keyed by DRAM tensor name."""
    tokens = np.asarray(inputs['tokens'])[:t_steps]
    emb_W = np.asarray(inputs['emb_W'], np.float32)
    dec_W = np.asarray(inputs['dec_W'], np.float32)

    x = emb_W[tokens.reshape(-1)]                    # [T*B, 400]
    xT = np.zeros((EP, t_steps * B), np.float32)
    xT[:E] = x.T
    xT_f = _fold_k(xT.astype(BF16), NKE)             # [128, 4, 5600]

    h0i = _full_hT(inputs['h0_0'], H, HP, NKH)
    h1i = _full_hT(inputs['h0_1'], H, HP, NKH)
    h2i = _full_hT(inputs['h0_2'], E, H2P, NKH2)

    Ws = []
    for l, (in_sz, hsz) in enumerate([(E, H), (H, H), (H, E)]):
        Ws.append((np.asarray(inputs[f'Wih{l}'], np.float32),
                   np.asarray(inputs[f'Whh{l}'], np.float32),
                   np.asarray(inputs[f'bih{l}'], np.float32),
                   np.asarray(inputs[f'bhh{l}'], np.float32)))

    in_maps = []
    for k in range(NCORES):
        rm0 = _gate_row_map_l01(k)
        w0i, w0h, b0 = _make_wT(Ws[0][0], Ws[0][1], Ws[0][2], Ws[0][3], k, H, rm0, E, EP, HP)
        w1i, w1h, b1 = _make_wT(Ws[1][0], Ws[1][1], Ws[1][2], Ws[1][3], k, H, rm0, H, HP, HP)
        rm2 = _gate_row_map_l2(k)
        w2i, w2h, b2 = _make_wT(Ws[2][0], Ws[2][1], Ws[2][2], Ws[2][3], k, E, rm2, H, HP, H2P)

        dwt = np.zeros((H2P, VS), np.float32)
        vlo, vhi = VS * k, min(V, VS * (k + 1))
        dwt[:E, :vhi - vlo] = dec_W[vlo:vhi].T

        m = {
            'xT': xT_f,
            'w0i': _fold_k(w0i, NKE), 'w0h': _fold_k(w0h, NKH), 'b0': _bias_tiles_l01(b0),
            'w1i': _fold_k(w1i, NKH), 'w1h': _fold_k(w1h, NKH),
            'b1': b1.reshape(1, 640).astype(BF16),
            'w2i': _fold_k(w2i, NKH), 'w2h': _fold_k(w2h, NKH2),
            'b2': b2.reshape(1, 256).astype(BF16),
            'dwt': _fold_k(dwt.astype(BF16), NKH2),
            'h0i': h0i, 'h1i': h1i, 'h2i': h2i,
            'c0i': _c_pack_l01(inputs['c0_0'], k),
            'c1i': _c_pack_l01(inputs['c0_1'], k),
            'c2i': _c_pack_l2(inputs['c0_2'], k),
        }
        in_maps.append(m)
    return in_maps


# ----------------------------------------------------------------------------
# device program
# ----------------------------------------------------------------------------

def build_program(t_steps=T, n_cores=NCORES):
    import concourse.bass as bass
    import concourse.bacc as bacc
    import concourse.mybir as mybir
    import concourse.tile as tile

    f32 = mybir.dt.float32
    bf16 = mybir.dt.bfloat16
    AF = mybir.ActivationFunctionType
    ALU = mybir.AluOpType

    TB = t_steps * B
    n_chunks = (t_steps + TCH - 1) // TCH

    nc = bacc.Bacc("TRN2", target_bir_lowering=False, debug=False,
                   num_devices=n_cores)

    # ---- I/O -------------------------------------------------------------
    xT_d = nc.dram_tensor('xT', [128, NKE, TB], bf16, kind="ExternalInput")
    w0i_d = nc.dram_tensor('w0i', [128, NKE, 640], bf16, kind="ExternalInput")
    w0h_d = nc.dram_tensor('w0h', [128, NKH, 640], bf16, kind="ExternalInput")
    b0_d = nc.dram_tensor('b0', [128, 5], f32, kind="ExternalInput")
    w1i_d = nc.dram_tensor('w1i', [128, NKH, 640], bf16, kind="ExternalInput")
    w1h_d = nc.dram_tensor('w1h', [128, NKH, 640], bf16, kind="ExternalInput")
    b1_d = nc.dram_tensor('b1', [1, 640], bf16, kind="ExternalInput")
    w2i_d = nc.dram_tensor('w2i', [128, NKH, 256], bf16, kind="ExternalInput")
    w2h_d = nc.dram_tensor('w2h', [128, NKH2, 256], bf16, kind="ExternalInput")
    b2_d = nc.dram_tensor('b2', [1, 256], bf16, kind="ExternalInput")
    dwt_d = nc.dram_tensor('dwt', [128, NKH2, VS], bf16, kind="ExternalInput")
    h0i_d = nc.dram_tensor('h0i', [128, NKH, B], bf16, kind="ExternalInput")
    h1i_d = nc.dram_tensor('h1i', [128, NKH, B], bf16, kind="ExternalInput")
    h2i_d = nc.dram_tensor('h2i', [128, NKH2, B], bf16, kind="ExternalInput")
    c0i_d = nc.dram_tensor('c0i', [128, 2, B], f32, kind="ExternalInput")
    c1i_d = nc.dram_tensor('c1i', [128, 2, B], f32, kind="ExternalInput")
    c2i_d = nc.dram_tensor('c2i', [128, B], f32, kind="ExternalInput")

    dec_d = nc.dram_tensor('dec', [TB, VS], f32, kind="ExternalOutput")
    hT0_d = nc.dram_tensor('hT0', [128, 2, B], f32, kind="ExternalOutput")
    cT0_d = nc.dram_tensor('cT0', [128, 2, B], f32, kind="ExternalOutput")
    hT1_d = nc.dram_tensor('hT1', [128, 2, B], f32, kind="ExternalOutput")
    cT1_d = nc.dram_tensor('cT1', [128, 2, B], f32, kind="ExternalOutput")
    hT2_d = nc.dram_tensor('hT2', [128, B], f32, kind="ExternalOutput")
    cT2_d = nc.dram_tensor('cT2', [128, B], f32, kind="ExternalOutput")

    # internal DRAM: layer0 input contributions.
    # M-groups: cols 0:4 = lo gates (128 rows), col 4 = hi combo [i f o g]x32.
    xw0_d = nc.dram_tensor('xw0', [128, 5, TB], f32)

    rg = [list(range(n_cores))]

    with tile.TileContext(nc) as tc:
        with (
            tc.tile_pool(name="wpool", bufs=1) as wpool,
            tc.tile_pool(name="ys", bufs=1) as yspool,
            tc.tile_pool(name="state", bufs=1) as stpool,
            tc.tile_pool(name="xtp", bufs=2) as xtp,
            tc.tile_pool(name="hx", bufs=3) as hxp,
            tc.tile_pool(name="work", bufs=2) as wk,
            tc.tile_pool(name="hout", bufs=3) as hop,
            tc.tile_pool(name="xw0sb", bufs=3) as xw0p,
            tc.tile_pool(name="psg0", bufs=1, space="PSUM") as psg0,
            tc.tile_pool(name="psg1", bufs=1, space="PSUM") as psg1,
            tc.tile_pool(name="psg2", bufs=1, space="PSUM") as psg2,
            tc.tile_pool(name="psx", bufs=1, space="PSUM") as psx,
            tc.tile_pool(name="psd", bufs=2, space="PSUM") as psd,
            tc.tile_pool(name="dram", bufs=3, space="DRAM") as dpool,
            tc.tile_pool(name="dramsh", bufs=3, space="DRAM") as dshpool,
        ):
            # ---- resident weights ----------------------------------------
            w0i = wpool.tile([128, NKE, 640], bf16)
            w0h = wpool.tile([128, NKH, 640], bf16)
            w1i = wpool.tile([128, NKH, 640], bf16)
            w1h = wpool.tile([128, NKH, 640], bf16)
            w2i = wpool.tile([128, NKH, 256], bf16)
            w2h = wpool.tile([128, NKH2, 256], bf16)
            dwt = wpool.tile([128, NKH2, VS], bf16)
            b0 = wpool.tile([128, 5], f32)
            b1 = wpool.tile([1, 640], bf16, padded_shape=[128, 640])
            b2 = wpool.tile([1, 256], bf16, padded_shape=[128, 256])
            ones = wpool.tile([1, B], bf16, padded_shape=[128, B])
            nc.gpsimd.memset(ones[0:1, :], 1.0)
            for sb, dr in [(w0i, w0i_d), (w0h, w0h_d), (w1i, w1i_d), (w1h, w1h_d),
                           (w2i, w2i_d), (w2h, w2h_d), (dwt, dwt_d),
                           (b0, b0_d), (b1, b1_d), (b2, b2_d)]:
                nc.scalar.dma_start(out=sb[:], in_=dr[:])

            # ys history (layer-2 h over all time) — decode lhsT
            ysT = yspool.tile([128, NKH2, TB], bf16)

            # persistent cell state
            c0 = stpool.tile([128, 2, B], f32)
            c1 = stpool.tile([128, 2, B], f32)
            c2 = stpool.tile([128, B], f32)
            nc.sync.dma_start(out=c0[:], in_=c0i_d[:])
            nc.sync.dma_start(out=c1[:], in_=c1i_d[:])
            nc.sync.dma_start(out=c2[:], in_=c2i_d[:])

            # initial h (full, replicated)
            h0_init = stpool.tile([128, NKH, B], bf16)
            h1_init = stpool.tile([128, NKH, B], bf16)
            h2_init = stpool.tile([128, NKH2, B], bf16)
            nc.sync.dma_start(out=h0_init[:], in_=h0i_d[:])
            nc.sync.dma_start(out=h1_init[:], in_=h1i_d[:])
            nc.sync.dma_start(out=h2_init[:], in_=h2i_d[:])

            # rolling full-h tiles (from AllGather)
            h0T = [h0_init]   # h0T[t] = gathered h0 after step t-? ; index offset 1
            h1T = [h1_init]
            hsb_last = {}

            # ---- helpers --------------------------------------------------
            def emit_xw0_chunk(c):
                """xw0[:, :, cols] = w0i.T @ xT[:, :, cols] for time-chunk c."""
                lo = c * TCH * B
                hi = min(TB, (c + 1) * TCH * B)
                ncols = hi - lo
                xt = xtp.tile([128, NKE, ncols], bf16, name=f"xt{c}",
                              tag="xt", bufs=1, padded_shape=[128, NKE, TCH * B])
                nc.scalar.dma_start(out=xt[:], in_=xT_d[:, :, lo:hi])
                for m in range(5):
                    mw, mc = 128, m * 128
                    for off in range(0, ncols, 512):
                        nw = min(512, ncols - off)
                        ps = psx.tile([128, 512], f32, name=f"psx{c}_{m}_{off}", tag="psx")
                        for kt in range(NKE):
                            nc.tensor.matmul(ps[:mw, :nw],
                                             w0i[:, kt, mc:mc + mw],
                                             xt[:, kt, off:off + nw],
                                             start=(kt == 0), stop=(kt == NKE - 1))
                        xsb = xtp.tile([128, 512], f32, name=f"xsb{c}_{m}_{off}",
                                       tag="xsb", bufs=2)
                        nc.vector.tensor_copy(xsb[:mw, :nw], ps[:mw, :nw])
                        nc.scalar.dma_start(out=xw0_d[:mw, m, lo + off:lo + off + nw],
                                            in_=xsb[:mw, :nw])

            def lstm_step_l01(l, t, wh, wi, bias, hT_list, other_hT, cst, psg, agin):
                """One step of layer 0 or 1.

                M-groups are gate-pure: ps_lo[:, g, :] holds gate g rows 0:128
                of this core's slice; ps_hi[:32, g, :] holds rows 128:160.
                Gate order g: 0=i 1=f 2=o 3=g.
                """
                ps_lo = psg.tile([128, 4, B], f32, name=f"pslo{l}_{t}", tag=f"pslo{l}")
                ps_hi = psg.tile([128, B], f32, name=f"pshi{l}_{t}", tag=f"pshi{l}")
                hprev = hT_list[t]  # [128, NKH, B]  (hT_list[t] = h_{t-1})
                nmm = NKH + (NKH + 1 if l == 1 else 0)
                for g in range(5):
                    # g<4: lo gate groups; g==4: hi combo [i f o g]x32
                    ps = ps_lo[:, g, :] if g < 4 else ps_hi[:, :]
                    mc = g * 128
                    idx = 0
                    for kt in range(NKH):
                        nc.tensor.matmul(ps, wh[:, kt, mc:mc + 128],
                                         hprev[:, kt, :],
                                         start=(idx == 0), stop=(idx == nmm - 1))
                        idx += 1
                    if l == 1:
                        for kt in range(NKH):
                            nc.tensor.matmul(ps, wi[:, kt, mc:mc + 128],
                                             other_hT[:, kt, :],
                                             start=False, stop=False)
                            idx += 1
                        # bias via K=1 matmul: bias_row.T @ ones
                        nc.tensor.matmul(ps, bias[0:1, mc:mc + 128], ones[0:1, :],
                                         start=False, stop=True)

                acts_lo = wk.tile([128, 4, B], f32, name=f"aclo{l}_{t}", tag=f"aclo{l}")
                acts_hi = wk.tile([32, 4, B], f32, name=f"achi{l}_{t}", tag=f"achi{l}")
                if l == 0:
                    # gates = psum + bias + xw0_t ; then activations
                    xw = xw0p.tile([128, 5, B], f32, name=f"xw0t{t}", tag="xw0t")
                    nc.scalar.dma_start(out=xw[:], in_=xw0_d[:, :, t * B:(t + 1) * B])
                    glo = wk.tile([128, 4, B], f32, name=f"glo{l}_{t}", tag=f"glo{l}")
                    ghi = wk.tile([128, B], f32, name=f"ghi{l}_{t}", tag=f"ghi{l}")
                    for g in range(4):
                        nc.vector.scalar_tensor_tensor(
                            glo[:, g, :], ps_lo[:, g, :], bias[:, g:g + 1],
                            xw[:, g, :], op0=ALU.add, op1=ALU.add)
                    nc.vector.scalar_tensor_tensor(
                        ghi[:, :], ps_hi[:, :], bias[:, 4:5],
                        xw[:, 4, :], op0=ALU.add, op1=ALU.add)
                    nc.scalar.activation(acts_lo[:, 0:3, :], glo[:, 0:3, :], AF.Sigmoid)
                    nc.scalar.activation(acts_lo[:, 3, :], glo[:, 3, :], AF.Tanh)
                    # hi combo: per-gate activation with realign to partition base 0
                    for g in range(3):
                        nc.scalar.activation(acts_hi[:32, g, :],
                                             ghi[32 * g:32 * g + 32, :], AF.Sigmoid)
                    nc.scalar.activation(acts_hi[:32, 3, :], ghi[96:128, :], AF.Tanh)
                else:
                    # bias already accumulated in PSUM (K=1 matmul)
                    nc.scalar.activation(acts_lo[:, 0:3, :], ps_lo[:, 0:3, :], AF.Sigmoid)
                    nc.scalar.activation(acts_lo[:, 3, :], ps_lo[:, 3, :], AF.Tanh)
                    for g in range(3):
                        nc.scalar.activation(acts_hi[:32, g, :],
                                             ps_hi[32 * g:32 * g + 32, :], AF.Sigmoid)
                    nc.scalar.activation(acts_hi[:32, 3, :], ps_hi[96:128, :], AF.Tanh)

                # c update: c = sig(f)*c + sig(i)*tanh(g); h = sig(o)*tanh(c)
                tmp = wk.tile([128, 2, B], f32, name=f"tmp{l}_{t}", tag=f"tmp{l}")
                tmph = wk.tile([32, 2, B], f32, name=f"tmph{l}_{t}", tag=f"tmph{l}")
                tc_t = wk.tile([128, 2, B], f32, name=f"tc{l}_{t}", tag=f"tc{l}")
                h_sb = hop.tile([128, 2, B], bf16, name=f"h{l}_{t}", tag=f"h{l}")
                # lo
                nc.vector.tensor_mul(tmp[:, 0, :], acts_lo[:, 1, :], cst[:, 0, :])
                nc.vector.tensor_mul(tmp[:, 1, :], acts_lo[:, 0, :], acts_lo[:, 3, :])
                nc.vector.tensor_add(cst[:, 0, :], tmp[:, 0, :], tmp[:, 1, :])
                nc.scalar.activation(tc_t[:, 0, :], cst[:, 0, :], AF.Tanh)
                nc.vector.tensor_mul(h_sb[:, 0, :], acts_lo[:, 2, :], tc_t[:, 0, :])
                # hi (32 rows, all at partition base 0)
                nc.vector.tensor_mul(tmph[:32, 0, :], acts_hi[:32, 1, :], cst[:32, 1, :])
                nc.vector.tensor_mul(tmph[:32, 1, :], acts_hi[:32, 0, :], acts_hi[:32, 3, :])
                nc.vector.tensor_add(cst[:32, 1, :], tmph[:32, 0, :], tmph[:32, 1, :])
                nc.scalar.activation(tc_t[:32, 1, :], cst[:32, 1, :], AF.Tanh)
                nc.vector.tensor_mul(h_sb[:32, 1, :], acts_hi[:32, 2, :], tc_t[:32, 1, :])

                # write this layer's slice into the combined AllGather input
                off = l * SL
                nc.sync.dma_start(out=agin[off:off + 128, :], in_=h_sb[:, 0, :])
                nc.sync.dma_start(out=agin[off + 128:off + SL, :], in_=h_sb[:32, 1, :])
                hsb_last[l] = h_sb
                return h_sb

            def lstm_step_l2(t, h1T_t, agin):
                # M-groups of 128: m0 = [i(0:64)|f(64:128)], m1 = [o|g]
                ps = psg2.tile([128, 2, B], f32, name=f"ps2_{t}", tag="ps2")
                for m in range(2):
                    idx = 0
                    for kt in range(NKH2):
                        rhs = ysT[:, kt, (t - 1) * B:t * B] if t > 0 else h2_init[:, kt, :]
                        nc.tensor.matmul(ps[:, m, :], w2h[:, kt, m * 128:(m + 1) * 128],
                                         rhs, start=(idx == 0), stop=False)
                        idx += 1
                    for kt in range(NKH):
                        nc.tensor.matmul(ps[:, m, :], w2i[:, kt, m * 128:(m + 1) * 128],
                                         h1T_t[:, kt, :], start=False, stop=False)
                        idx += 1
                    nc.tensor.matmul(ps[:, m, :], b2[0:1, m * 128:(m + 1) * 128],
                                     ones[0:1, :], start=False, stop=True)
                acts = wk.tile([64, 4, B], f32, name=f"acts2_{t}", tag="acts2")
                # realign to base 0: acts cols 0=i 1=f 2=o 3=g
                nc.scalar.activation(acts[:64, 0, :], ps[0:64, 0, :], AF.Sigmoid)
                nc.scalar.activation(acts[:64, 1, :], ps[64:128, 0, :], AF.Sigmoid)
                nc.scalar.activation(acts[:64, 2, :], ps[0:64, 1, :], AF.Sigmoid)
                nc.scalar.activation(acts[:64, 3, :], ps[64:128, 1, :], AF.Tanh)
                tmp = wk.tile([64, 2, B], f32, name=f"tmp2_{t}", tag="tmp2")
                tc_t = wk.tile([64, B], f32, name=f"tc2_{t}", tag="tc2")
                h_sb = hop.tile([64, B], bf16, name=f"h2_{t}", tag="h2")
                nc.vector.tensor_mul(tmp[:64, 0, :], acts[:64, 1, :], c2[:64, :])
                nc.vector.tensor_mul(tmp[:64, 1, :], acts[:64, 0, :], acts[:64, 3, :])
                nc.vector.tensor_add(c2[:64, :], tmp[:64, 0, :], tmp[:64, 1, :])
                nc.scalar.activation(tc_t[:64, :], c2[:64, :], AF.Tanh)
                nc.vector.tensor_mul(h_sb[:64, :], acts[:64, 2, :], tc_t[:64, :])

                nc.sync.dma_start(out=agin[2 * SL:2 * SL + SL2, :], in_=h_sb[:64, :])
                hsb_last[2] = h_sb

            def emit_decode_chunk(c):
                lo = c * TCH * B
                hi = min(TB, (c + 1) * TCH * B)
                ncols = hi - lo
                for m in range((ncols + 127) // 128):
                    mw = min(128, ncols - m * 128)
                    for half in range(2):
                        # stage half a vocab row [2080 cols] in SBUF, 1 DMA out
                        hoff = half * (VS // 2)
                        dst = wk.tile([128, VS // 2], f32, name=f"dst{c}_{m}_{half}",
                                      tag="dst", bufs=2)
                        for off in range(hoff, hoff + VS // 2, 512):
                            nw = min(512, hoff + VS // 2 - off)
                            ps = psd.tile([128, 512], f32, name=f"psd{c}_{m}_{off}",
                                          tag="psd")
                            for kt in range(NKH2):
                                nc.tensor.matmul(
                                    ps[:mw, :nw],
                                    ysT[:, kt, lo + m * 128: lo + m * 128 + mw],
                                    dwt[:, kt, off:off + nw],
                                    start=(kt == 0), stop=(kt == NKH2 - 1))
                            nc.vector.tensor_copy(dst[:mw, off - hoff:off - hoff + nw],
                                                  ps[:mw, :nw])
                        nc.scalar.dma_start(
                            out=dec_d[lo + m * 128: lo + m * 128 + mw,
                                      hoff:hoff + VS // 2],
                            in_=dst[:mw, :])

            # ---- main wavefront loop -------------------------------------
            # combined AllGather block per rank: [h0(160) h1(160) h2(64)] = 384 rows
            AGB = 2 * SL + SL2   # 384
            zero384 = stpool.tile([128, 3, B], bf16, name="zero384")
            nc.gpsimd.memset(zero384[:], 0.0)

            emit_xw0_chunk(0)
            n_slots = t_steps + 2
            for s in range(n_slots):
                # emit xw0 chunk c at slot 8*(c-1)+3 (5 slots before first use)
                if s >= 3 and (s - 3) % TCH == 0:
                    cdue = (s - 3) // TCH + 1
                    if cdue < n_chunks:
                        emit_xw0_chunk(cdue)

                agin = dpool.tile([AGB, B], bf16, name=f"agi_{s}", tag="agi")

                # zero-fill slices of layers not active this slot (pipeline edges)
                def zero_fill(a, b2):
                    while a < b2:
                        n = min(128, b2 - a)
                        nc.sync.dma_start(out=agin[a:a + n, :], in_=zero384[:n, 0, :])
                        a += n
                if not (s < t_steps):
                    zero_fill(0, SL)
                if not (0 <= s - 1 < t_steps):
                    zero_fill(SL, 2 * SL)
                if not (0 <= s - 2 < t_steps):
                    zero_fill(2 * SL, AGB)

                if s < t_steps:
                    lstm_step_l01(0, s, w0h, w0i, b0, h0T, None, c0, psg0, agin)
                if 0 <= s - 1 < t_steps:
                    t = s - 1
                    lstm_step_l01(1, t, w1h, w1i, b1, h1T, h0T[t + 1], c1, psg1, agin)
                if 0 <= s - 2 < t_steps:
                    t = s - 2
                    lstm_step_l2(t, h1T[t + 1], agin)

                # one AllGather per slot
                agout = dshpool.tile([NCORES * AGB, B], bf16, name=f"ago_{s}",
                                     tag="ago", addr_space="Shared")
                nc.gpsimd.collective_compute(
                    "AllGather", ALU.bypass, ins=[agin.opt()], outs=[agout.opt()],
                    replica_groups=rg)

                # scatter gathered blocks back into K-tile layout
                # view (k-major): A[r, k, b] = agout[k*AGB + r, b]
                A = agout.rearrange("(k r) b -> r k b", r=AGB)
                # view for 32-row leftovers: Aq[r, c, q, b] = agout[(c*4+q)*AGB + r, b]
                Aq = agout.rearrange("(c q r) b -> r c q b", c=2, q=4)
                # view for layer-2 64-row halves: Ah[r, kt, q, b] = agout[(kt*2+q)*AGB + r, b]
                Ah = agout.rearrange("(kt q r) b -> r kt q b", kt=4, q=2)
                if s < t_steps:
                    hT_new = hxp.tile([128, NKH, B], bf16, name=f"hT0_{s}", tag="hT0")
                    nc.sync.dma_start(out=hT_new[:, 0:8, :], in_=A[0:128, :, :])
                    for q in range(4):
                        nc.sync.dma_start(out=hT_new[32 * q:32 * q + 32, 8:10, :],
                                          in_=Aq[128:160, :, q, :])
                    h0T.append(hT_new)
                if 0 <= s - 1 < t_steps:
                    hT_new = hxp.tile([128, NKH, B], bf16, name=f"hT1_{s}", tag="hT1")
                    nc.sync.dma_start(out=hT_new[:, 0:8, :], in_=A[160:288, :, :])
                    for q in range(4):
                        nc.sync.dma_start(out=hT_new[32 * q:32 * q + 32, 8:10, :],
                                          in_=Aq[288:320, :, q, :])
                    h1T.append(hT_new)
                if 0 <= s - 2 < t_steps:
                    t = s - 2
                    for q in range(2):
                        nc.sync.dma_start(out=ysT[64 * q:64 * q + 64, :, t * B:(t + 1) * B],
                                          in_=Ah[320:384, :, q, :])

                # decode chunk c done when L2 step (c+1)*TCH-1 finished at slot +2
                if s >= 10 and (s - 10) % TCH == 0 and (s - 10) // TCH < n_chunks:
                    emit_decode_chunk((s - 10) // TCH)
            # remaining decode chunks
            first_rem = (n_slots - 10) // TCH + 1 if n_slots >= 10 else 0
            for c in range(max(0, first_rem), n_chunks):
                emit_decode_chunk(c)

            # ---- final states --------------------------------------------
            for l, (hd, cd, cst) in enumerate([(hT0_d, cT0_d, c0), (hT1_d, cT1_d, c1)]):
                f = wk.tile([128, 2, B], f32, name=f"fin{l}", tag="fin", bufs=2)
                nc.gpsimd.memset(f[:], 0.0)
                nc.vector.tensor_copy(f[:, 0, :], hsb_last[l][:, 0, :])
                nc.vector.tensor_copy(f[:32, 1, :], hsb_last[l][:32, 1, :])
                nc.sync.dma_start(out=hd[:], in_=f[:])
                nc.sync.dma_start(out=cd[:], in_=cst[:])
            f2 = wk.tile([128, B], f32, name="fin2", tag="fin2")
            nc.gpsimd.memset(f2[:], 0.0)
            nc.vector.tensor_copy(f2[:64, :], hsb_last[2][:64, :])
            nc.sync.dma_start(out=hT2_d[:], in_=f2[:])
            nc.sync.dma_start(out=cT2_d[:], in_=c2[:])

    nc.compile()
    return nc


# ----------------------------------------------------------------------------
# entry point
# ----------------------------------------------------------------------------

_cached = {}


def _get_program(t_steps=T):
    if t_steps not in _cached:
        _cached[t_steps] = build_program(t_steps)
    return _cached[t_steps]


def run(inputs, t_steps=T, trace=False, trace_kwargs=None):
    from concourse.bass_utils import run_bass_kernel_spmd
    nc = _get_program(t_steps)
    in_maps = prepare_inputs(inputs, t_steps)
    res = run_bass_kernel_spmd(nc, in_maps, core_ids=list(range(NCORES)),
                               trace=trace, **(trace_kwargs or {}))
    return res


def assemble_outputs(results, inputs, t_steps=T):
    TB_ = t_steps * B
    decoded = np.empty((t_steps, B, V), np.float32)
    for k in range(NCORES):
        d = results[k]['dec'].reshape(t_steps, B, VS)
        vlo, vhi = VS * k, min(V, VS * (k + 1))
        decoded[:, :, vlo:vhi] = d[:, :, :vhi - vlo]
    dec_b = np.asarray(inputs['dec_b'], np.float32)
    if np.any(dec_b):
        decoded += dec_b

    def unpack_l01(key, hsz):
        out = np.zeros((B, hsz), np.float32)
        for k in range(NCORES):
            a = results[k][key]          # [128, 2, B]
            sl = np.concatenate([a[:, 0, :], a[:32, 1, :]], axis=0)  # [160, B]
            for i, j in enumerate(_slice_rows_l01(k)):
                if j < hsz:
                    out[:, j] = sl[i]
        return out

    def unpack_l2(key, hsz):
        out = np.zeros((B, hsz), np.float32)
        for k in range(NCORES):
            a = results[k][key]          # [128, B]
            lo, hi = SL2 * k, min(hsz, SL2 * (k + 1))
            if hi > lo:
                out[:, lo:hi] = a[:hi - lo].T
        return out

    hT0 = unpack_l01('hT0', H); cT0 = unpack_l01('cT0', H)
    hT1 = unpack_l01('hT1', H); cT1 = unpack_l01('cT1', H)
    hT2 = unpack_l2('hT2', E); cT2 = unpack_l2('cT2', E)
    return (decoded, hT0, cT0, hT1, cT1, hT2, cT2)


def kernel(**inputs):
    res = run(inputs, t_steps=T)
    return assemble_outputs(res.results, inputs, t_steps=T)


# revision 61
# speedup vs baseline: 1.0513x; 1.0513x over previous
"""AWD-LSTM forward on 8 Trainium2 NeuronCores (Bass/Tile, SPMD).

Strategy:
  - 8-way tensor parallelism over the hidden/gate dimension for all three LSTM
    layers (H=1150 -> padded 1280, 160 rows/core; layer2 H=400 -> 512, 64/core),
    with a per-step AllGather of h (bf16).
  - Everything lives in transposed layout: features on partitions, batch (80)
    on the free dim.  Per-step gates G.T[:, b] accumulate in PSUM from
    (a) Whh_slice.T.T @ h.T (recurrent) and, for layers 1/2, (b) Wih_slice @ x.T
    fused into the same accumulation group.  Layer0's input contribution is a
    bulk GEMM from the (host-gathered) embedding, staged through DRAM.
  - Vocab projection is sharded over V (33280/8 = 4160 cols/core), computed in
    time-chunks from the accumulated ys.T history, PSUM DMA'd straight to HBM.
  - Emission follows the wavefront (slot s: L0 step s, L1 step s-1, L2 step
    s-2, due xw0/decode chunks) so each engine's in-order stream interleaves
    layers and the AllGather latency is hidden by compute of other layers.
  - bf16 matmuls, fp32 cell state & PSUM accumulation.
"""
import os
import sys

sys.path.insert(0, '/opt/trn_rl_repo')

import numpy as np
import ml_dtypes

BF16 = ml_dtypes.bfloat16

# problem shapes (hardcoded per contract)
V, E, H, T, B = 33278, 400, 1150, 70, 80
NCORES = 8
EP = 512            # padded E (K dim)
HP = 1280           # padded H (layers 0,1)
SL = HP // NCORES   # 160 hidden rows per core
H2P = 512           # padded layer-2 H (=400)
SL2 = H2P // NCORES  # 64
VP = 33280
VS = VP // NCORES   # 4160
NKE = EP // 128     # 4   K-tiles over embedding dim
NKH = HP // 128     # 10  K-tiles over H (layers 0,1)
NKH2 = H2P // 128   # 4   K-tiles over layer-2 H
NM01 = 5            # M-tiles of per-core gates, layers 0/1 (640 rows)
NM2 = 2             # M-tiles of per-core gates, layer 2 (256 rows)
TCH = 8             # decode/xw0 time-chunk (steps)


# ----------------------------------------------------------------------------
# host-side prep
# ----------------------------------------------------------------------------

def _slice_rows_l01(k):
    """Core k's hidden rows (padded H=1280): a 128-block plus a 32-block.

    This split makes the combined-AllGather output (per-rank blocks of
    [h0(128+32) h1(128+32) h2(64)]) map back onto plain contiguous 128-row
    K-tiles with affine DMA access patterns.
    """
    return list(range(128 * k, 128 * (k + 1))) + \
        list(range(1024 + 32 * k, 1024 + 32 * (k + 1)))


def _gate_row_map_l01(k):
    # M order: [i_lo(128) f_lo(128) o_lo(128) g_lo(128) | i_hi f_hi o_hi g_hi (32 each)]
    # orig (PyTorch) gate order: i=0 f=1 g=2 o=3;  ours [i,f,o,g] -> [0,1,3,2]
    order = [0, 1, 3, 2]
    sr = _slice_rows_l01(k)
    rows = []
    for m in range(640):
        if m < 512:
            gate = order[m // 128]
            j = sr[m % 128]
        else:
            sub = m - 512
            gate = order[sub // 32]
            j = sr[128 + (sub % 32)]
        rows.append((gate, j))
    return rows


def _gate_row_map_l2(k):
    order = [0, 1, 3, 2]
    return [(order[m // 64], SL2 * k + (m % 64)) for m in range(256)]


def _make_wT(Wih, Whh, bih, bhh, k, hsz, row_map, in_sz, in_pad, hpad):
    nM = len(row_map)
    wi = np.zeros((in_pad, nM), np.float32)
    wh = np.zeros((hpad, nM), np.float32)
    b = np.zeros(nM, np.float32)
    for m, (gate, j) in enumerate(row_map):
        if j < hsz:
            r = gate * hsz + j
            wi[:in_sz, m] = Wih[r]
            wh[:hsz, m] = Whh[r]
            b[m] = bih[r] + bhh[r]
    return wi.astype(BF16), wh.astype(BF16), b


def _fold_k(a, nk):
    """[nk*128, N] -> [128, nk, N] with row r=(kt*128+p) -> [p, kt]."""
    n = a.shape[1]
    return np.ascontiguousarray(a.reshape(nk, 128, n).transpose(1, 0, 2))


def _bias_tiles_l01(b):
    # b: [640] (4x128 lo + 4x32 hi) -> [128, 5]: cols 0:4 lo gates, col 4 = hi combo
    out = np.zeros((128, 5), np.float32)
    for g in range(4):
        out[:, g] = b[g * 128:(g + 1) * 128]
        out[32 * g:32 * (g + 1), 4] = b[512 + g * 32: 512 + (g + 1) * 32]
    return out


def _bias_tiles_l2(b):
    # b: [256] (4x64, order i f o g) -> [128, 2]: col0 = [i|f], col1 = [o|g]
    out = np.zeros((128, 2), np.float32)
    out[:64, 0] = b[0:64]
    out[64:, 0] = b[64:128]
    out[:64, 1] = b[128:192]
    out[64:, 1] = b[192:256]
    return out


def _full_hT(h0, hsz, hpad, nk):
    out = np.zeros((hpad, B), np.float32)
    out[:hsz] = np.asarray(h0, np.float32).T
    return _fold_k(out.astype(BF16), nk)


def _c_pack_l01(c0, k):
    # per-core c slice -> [128, 2, 80] (col0 = 128-block, col1[:32] = 32-block)
    out = np.zeros((128, 2, B), np.float32)
    c0 = np.asarray(c0, np.float32)
    sr = _slice_rows_l01(k)
    sl = np.zeros((SL, B), np.float32)
    for i, j in enumerate(sr):
        if j < H:
            sl[i] = c0[:, j]
    out[:, 0, :] = sl[:128]
    out[:32, 1, :] = sl[128:160]
    return out


def _c_pack_l2(c0, k):
    out = np.zeros((128, B), np.float32)
    lo, hi = SL2 * k, min(E, SL2 * (k + 1))
    if hi > lo:
        out[:hi - lo] = np.asarray(c0, np.float32)[:, lo:hi].T
    return out


def prepare_inputs(inputs, t_steps=T):
    """Returns in_maps: list of 8 dicts keyed by DRAM tensor name."""
    tokens = np.asarray(inputs['tokens'])[:t_steps]
    emb_W = np.asarray(inputs['emb_W'], np.float32)
    dec_W = np.asarray(inputs['dec_W'], np.float32)

    x = emb_W[tokens.reshape(-1)]                    # [T*B, 400]
    xT = np.zeros((EP, t_steps * B), np.float32)
    xT[:E] = x.T
    xT_f = _fold_k(xT.astype(BF16), NKE)             # [128, 4, 5600]

    h0i = _full_hT(inputs['h0_0'], H, HP, NKH)
    h1i = _full_hT(inputs['h0_1'], H, HP, NKH)
    h2i = _full_hT(inputs['h0_2'], E, H2P, NKH2)

    Ws = []
    for l, (in_sz, hsz) in enumerate([(E, H), (H, H), (H, E)]):
        Ws.append((np.asarray(inputs[f'Wih{l}'], np.float32),
                   np.asarray(inputs[f'Whh{l}'], np.float32),
                   np.asarray(inputs[f'bih{l}'], np.float32),
                   np.asarray(inputs[f'bhh{l}'], np.float32)))

    in_maps = []
    for k in range(NCORES):
        rm0 = _gate_row_map_l01(k)
        w0i, w0h, b0 = _make_wT(Ws[0][0], Ws[0][1], Ws[0][2], Ws[0][3], k, H, rm0, E, EP, HP)
        w1i, w1h, b1 = _make_wT(Ws[1][0], Ws[1][1], Ws[1][2], Ws[1][3], k, H, rm0, H, HP, HP)
        rm2 = _gate_row_map_l2(k)
        w2i, w2h, b2 = _make_wT(Ws[2][0], Ws[2][1], Ws[2][2], Ws[2][3], k, E, rm2, H, HP, H2P)

        dwt = np.zeros((H2P, VS), np.float32)
        vlo, vhi = VS * k, min(V, VS * (k + 1))
        dwt[:E, :vhi - vlo] = dec_W[vlo:vhi].T

        m = {
            'xT': xT_f,
            'w0i': _fold_k(w0i, NKE), 'w0h': _fold_k(w0h, NKH), 'b0': _bias_tiles_l01(b0),
            'w1i': _fold_k(w1i, NKH), 'w1h': _fold_k(w1h, NKH),
            'b1': b1.reshape(1, 640).astype(BF16),
            'w2i': _fold_k(w2i, NKH), 'w2h': _fold_k(w2h, NKH2),
            'b2': b2.reshape(1, 256).astype(BF16),
            'dwt': _fold_k(dwt.astype(BF16), NKH2),
            'h0i': h0i, 'h1i': h1i, 'h2i': h2i,
            'c0i': _c_pack_l01(inputs['c0_0'], k),
            'c1i': _c_pack_l01(inputs['c0_1'], k),
            'c2i': _c_pack_l2(inputs['c0_2'], k),
        }
        in_maps.append(m)
    return in_maps


# ----------------------------------------------------------------------------
# device program
# ----------------------------------------------------------------------------

def build_program(t_steps=T, n_cores=NCORES):
    import concourse.bass as bass
    import concourse.bacc as bacc
    import concourse.mybir as mybir
    import concourse.tile as tile

    f32 = mybir.dt.float32
    bf16 = mybir.dt.bfloat16
    AF = mybir.ActivationFunctionType
    ALU = mybir.AluOpType

    TB = t_steps * B
    n_chunks = (t_steps + TCH - 1) // TCH

    nc = bacc.Bacc("TRN2", target_bir_lowering=False, debug=False,
                   num_devices=n_cores)

    # ---- I/O -------------------------------------------------------------
    xT_d = nc.dram_tensor('xT', [128, NKE, TB], bf16, kind="ExternalInput")
    w0i_d = nc.dram_tensor('w0i', [128, NKE, 640], bf16, kind="ExternalInput")
    w0h_d = nc.dram_tensor('w0h', [128, NKH, 640], bf16, kind="ExternalInput")
    b0_d = nc.dram_tensor('b0', [128, 5], f32, kind="ExternalInput")
    w1i_d = nc.dram_tensor('w1i', [128, NKH, 640], bf16, kind="ExternalInput")
    w1h_d = nc.dram_tensor('w1h', [128, NKH, 640], bf16, kind="ExternalInput")
    b1_d = nc.dram_tensor('b1', [1, 640], bf16, kind="ExternalInput")
    w2i_d = nc.dram_tensor('w2i', [128, NKH, 256], bf16, kind="ExternalInput")
    w2h_d = nc.dram_tensor('w2h', [128, NKH2, 256], bf16, kind="ExternalInput")
    b2_d = nc.dram_tensor('b2', [1, 256], bf16, kind="ExternalInput")
    dwt_d = nc.dram_tensor('dwt', [128, NKH2, VS], bf16, kind="ExternalInput")
    h0i_d = nc.dram_tensor('h0i', [128, NKH, B], bf16, kind="ExternalInput")
    h1i_d = nc.dram_tensor('h1i', [128, NKH, B], bf16, kind="ExternalInput")
    h2i_d = nc.dram_tensor('h2i', [128, NKH2, B], bf16, kind="ExternalInput")
    c0i_d = nc.dram_tensor('c0i', [128, 2, B], f32, kind="ExternalInput")
    c1i_d = nc.dram_tensor('c1i', [128, 2, B], f32, kind="ExternalInput")
    c2i_d = nc.dram_tensor('c2i', [128, B], f32, kind="ExternalInput")

    dec_d = nc.dram_tensor('dec', [TB, VS], f32, kind="ExternalOutput")
    hT0_d = nc.dram_tensor('hT0', [128, 2, B], f32, kind="ExternalOutput")
    cT0_d = nc.dram_tensor('cT0', [128, 2, B], f32, kind="ExternalOutput")
    hT1_d = nc.dram_tensor('hT1', [128, 2, B], f32, kind="ExternalOutput")
    cT1_d = nc.dram_tensor('cT1', [128, 2, B], f32, kind="ExternalOutput")
    hT2_d = nc.dram_tensor('hT2', [128, B], f32, kind="ExternalOutput")
    cT2_d = nc.dram_tensor('cT2', [128, B], f32, kind="ExternalOutput")

    # internal DRAM: layer0 input contributions.
    # M-groups: cols 0:4 = lo gates (128 rows), col 4 = hi combo [i f o g]x32.
    xw0_d = nc.dram_tensor('xw0', [128, 5, TB], f32)

    rg = [list(range(n_cores))]

    with tile.TileContext(nc) as tc:
        with (
            tc.tile_pool(name="wpool", bufs=1) as wpool,
            tc.tile_pool(name="ys", bufs=1) as yspool,
            tc.tile_pool(name="state", bufs=1) as stpool,
            tc.tile_pool(name="xtp", bufs=2) as xtp,
            tc.tile_pool(name="hx", bufs=3) as hxp,
            tc.tile_pool(name="work", bufs=2) as wk,
            tc.tile_pool(name="hout", bufs=3) as hop,
            tc.tile_pool(name="xw0sb", bufs=3) as xw0p,
            tc.tile_pool(name="psg0", bufs=1, space="PSUM") as psg0,
            tc.tile_pool(name="psg1", bufs=1, space="PSUM") as psg1,
            tc.tile_pool(name="psg2", bufs=1, space="PSUM") as psg2,
            tc.tile_pool(name="psx", bufs=1, space="PSUM") as psx,
            tc.tile_pool(name="psd", bufs=2, space="PSUM") as psd,
            tc.tile_pool(name="dram", bufs=3, space="DRAM") as dpool,
            tc.tile_pool(name="dramsh", bufs=3, space="DRAM") as dshpool,
        ):
            # ---- resident weights ----------------------------------------
            w0i = wpool.tile([128, NKE, 640], bf16)
            w0h = wpool.tile([128, NKH, 640], bf16)
            w1i = wpool.tile([128, NKH, 640], bf16)
            w1h = wpool.tile([128, NKH, 640], bf16)
            w2i = wpool.tile([128, NKH, 256], bf16)
            w2h = wpool.tile([128, NKH2, 256], bf16)
            dwt = wpool.tile([128, NKH2, VS], bf16)
            b0 = wpool.tile([128, 5], f32)
            b1 = wpool.tile([1, 640], bf16, padded_shape=[128, 640])
            b2 = wpool.tile([1, 256], bf16, padded_shape=[128, 256])
            ones = wpool.tile([1, B], bf16, padded_shape=[128, B])
            nc.gpsimd.memset(ones[0:1, :], 1.0)
            for sb, dr in [(w0i, w0i_d), (w0h, w0h_d), (w1i, w1i_d), (w1h, w1h_d),
                           (w2i, w2i_d), (w2h, w2h_d), (dwt, dwt_d),
                           (b0, b0_d), (b1, b1_d), (b2, b2_d)]:
                nc.scalar.dma_start(out=sb[:], in_=dr[:])

            # ys history (layer-2 h over all time) — decode lhsT
            ysT = yspool.tile([128, NKH2, TB], bf16)

            # persistent cell state
            c0 = stpool.tile([128, 2, B], f32)
            c1 = stpool.tile([128, 2, B], f32)
            c2 = stpool.tile([128, B], f32)
            nc.sync.dma_start(out=c0[:], in_=c0i_d[:])
            nc.sync.dma_start(out=c1[:], in_=c1i_d[:])
            nc.sync.dma_start(out=c2[:], in_=c2i_d[:])

            # initial h (full, replicated)
            h0_init = stpool.tile([128, NKH, B], bf16)
            h1_init = stpool.tile([128, NKH, B], bf16)
            h2_init = stpool.tile([128, NKH2, B], bf16)
            nc.sync.dma_start(out=h0_init[:], in_=h0i_d[:])
            nc.sync.dma_start(out=h1_init[:], in_=h1i_d[:])
            nc.sync.dma_start(out=h2_init[:], in_=h2i_d[:])

            # rolling full-h entries (from AllGather).  Entry forms:
            #   ('full', tile[128, NKH, B])            -- initial state
            #   ('g', gath[128, 8, 3, B], hi[128, 2, B]) -- gathered slot
            h0T = [('full', h0_init)]   # h0T[t] = h_{t-1} for step t
            h1T = [('full', h1_init)]
            hsb_last = {}

            def rhs01(entry, x, kt):
                """K-tile kt of a layer-0/1 hidden state (x: 0=h0, 1=h1)."""
                if entry[0] == 'full':
                    return entry[1][:, kt, :]
                if kt < 8:
                    return entry[1][:, kt, x, :]
                return entry[2][:, kt - 8, :]

            # ---- helpers --------------------------------------------------
            def emit_xw0_chunk(c):
                """xw0[:, :, cols] = w0i.T @ xT[:, :, cols] for time-chunk c."""
                lo = c * TCH * B
                hi = min(TB, (c + 1) * TCH * B)
                ncols = hi - lo
                xt = xtp.tile([128, NKE, ncols], bf16, name=f"xt{c}",
                              tag="xt", bufs=1, padded_shape=[128, NKE, TCH * B])
                nc.scalar.dma_start(out=xt[:], in_=xT_d[:, :, lo:hi])
                for m in range(5):
                    mw, mc = 128, m * 128
                    for off in range(0, ncols, 512):
                        nw = min(512, ncols - off)
                        ps = psx.tile([128, 512], f32, name=f"psx{c}_{m}_{off}", tag="psx")
                        for kt in range(NKE):
                            nc.tensor.matmul(ps[:mw, :nw],
                                             w0i[:, kt, mc:mc + mw],
                                             xt[:, kt, off:off + nw],
                                             start=(kt == 0), stop=(kt == NKE - 1))
                        xsb = xtp.tile([128, 512], f32, name=f"xsb{c}_{m}_{off}",
                                       tag="xsb", bufs=2)
                        nc.vector.tensor_copy(xsb[:mw, :nw], ps[:mw, :nw])
                        nc.scalar.dma_start(out=xw0_d[:mw, m, lo + off:lo + off + nw],
                                            in_=xsb[:mw, :nw])

            def lstm_step_l01(l, t, wh, wi, bias, hT_list, other_hT, cst, psg, agin):
                """One step of layer 0 or 1.

                M-groups are gate-pure: ps_lo[:, g, :] holds gate g rows 0:128
                of this core's slice; ps_hi[:32, g, :] holds rows 128:160.
                Gate order g: 0=i 1=f 2=o 3=g.
                """
                ps_lo = psg.tile([128, 4, B], f32, name=f"pslo{l}_{t}", tag=f"pslo{l}")
                ps_hi = psg.tile([128, B], f32, name=f"pshi{l}_{t}", tag=f"pshi{l}")
                hprev = hT_list[t]  # entry; hT_list[t] = h_{t-1}
                nmm = NKH + (NKH + 1 if l == 1 else 0)
                for g in range(5):
                    # g<4: lo gate groups; g==4: hi combo [i f o g]x32
                    ps = ps_lo[:, g, :] if g < 4 else ps_hi[:, :]
                    mc = g * 128
                    idx = 0
                    for kt in range(NKH):
                        nc.tensor.matmul(ps, wh[:, kt, mc:mc + 128],
                                         rhs01(hprev, l, kt),
                                         start=(idx == 0), stop=(idx == nmm - 1))
                        idx += 1
                    if l == 1:
                        for kt in range(NKH):
                            nc.tensor.matmul(ps, wi[:, kt, mc:mc + 128],
                                             rhs01(other_hT, 0, kt),
                                             start=False, stop=False)
                            idx += 1
                        # bias via K=1 matmul: bias_row.T @ ones
                        nc.tensor.matmul(ps, bias[0:1, mc:mc + 128], ones[0:1, :],
                                         start=False, stop=True)

                acts_lo = wk.tile([128, 4, B], f32, name=f"aclo{l}_{t}", tag=f"aclo{l}")
                # hi combo activations (gate-major [i f o g]x32), then DVE copies
                # realign f/o/g to partition base 0
                hc = wk.tile([128, B], f32, name=f"hc{l}_{t}", tag=f"hc{l}")
                acts_hi = wk.tile([32, 3, B], f32, name=f"achi{l}_{t}", tag=f"achi{l}")
                if l == 0:
                    # gates = psum + bias + xw0_t ; then activations
                    xw = xw0p.tile([128, 5, B], f32, name=f"xw0t{t}", tag="xw0t")
                    nc.scalar.dma_start(out=xw[:], in_=xw0_d[:, :, t * B:(t + 1) * B])
                    glo = wk.tile([128, 4, B], f32, name=f"glo{l}_{t}", tag=f"glo{l}")
                    ghi = wk.tile([128, B], f32, name=f"ghi{l}_{t}", tag=f"ghi{l}")
                    for g in range(4):
                        nc.vector.scalar_tensor_tensor(
                            glo[:, g, :], ps_lo[:, g, :], bias[:, g:g + 1],
                            xw[:, g, :], op0=ALU.add, op1=ALU.add)
                    nc.vector.scalar_tensor_tensor(
                        ghi[:, :], ps_hi[:, :], bias[:, 4:5],
                        xw[:, 4, :], op0=ALU.add, op1=ALU.add)
                    nc.scalar.activation(acts_lo[:, 0:3, :], glo[:, 0:3, :], AF.Sigmoid)
                    nc.scalar.activation(acts_lo[:, 3, :], glo[:, 3, :], AF.Tanh)
                    nc.scalar.activation(hc[0:96, :], ghi[0:96, :], AF.Sigmoid)
                    nc.scalar.activation(hc[96:128, :], ghi[96:128, :], AF.Tanh)
                else:
                    # bias already accumulated in PSUM (K=1 matmul)
                    nc.scalar.activation(acts_lo[:, 0:3, :], ps_lo[:, 0:3, :], AF.Sigmoid)
                    nc.scalar.activation(acts_lo[:, 3, :], ps_lo[:, 3, :], AF.Tanh)
                    nc.scalar.activation(hc[0:96, :], ps_hi[0:96, :], AF.Sigmoid)
                    nc.scalar.activation(hc[96:128, :], ps_hi[96:128, :], AF.Tanh)
                # realign hi f/o/g to base 0 (cols 0=f 1=o 2=g); i read from hc[0:32]
                nc.vector.tensor_copy(acts_hi[:32, 0, :], hc[32:64, :])
                nc.vector.tensor_copy(acts_hi[:32, 1, :], hc[64:96, :])
                nc.vector.tensor_copy(acts_hi[:32, 2, :], hc[96:128, :])

                # c update: c = sig(f)*c + sig(i)*tanh(g); h = sig(o)*tanh(c)
                tmp = wk.tile([128, 2, B], f32, name=f"tmp{l}_{t}", tag=f"tmp{l}")
                tmph = wk.tile([32, 2, B], f32, name=f"tmph{l}_{t}", tag=f"tmph{l}")
                tc_t = wk.tile([128, 2, B], f32, name=f"tc{l}_{t}", tag=f"tc{l}")
                h_sb = hop.tile([128, 2, B], bf16, name=f"h{l}_{t}", tag=f"h{l}")
                # lo
                nc.vector.tensor_mul(tmp[:, 0, :], acts_lo[:, 1, :], cst[:, 0, :])
                nc.vector.tensor_mul(tmp[:, 1, :], acts_lo[:, 0, :], acts_lo[:, 3, :])
                nc.vector.tensor_add(cst[:, 0, :], tmp[:, 0, :], tmp[:, 1, :])
                nc.scalar.activation(tc_t[:, 0, :], cst[:, 0, :], AF.Tanh)
                nc.vector.tensor_mul(h_sb[:, 0, :], acts_lo[:, 2, :], tc_t[:, 0, :])
                # hi (32 rows, all at partition base 0)
                nc.vector.tensor_mul(tmph[:32, 0, :], acts_hi[:32, 0, :], cst[:32, 1, :])
                nc.vector.tensor_mul(tmph[:32, 1, :], hc[:32, :], acts_hi[:32, 2, :])
                nc.vector.tensor_add(cst[:32, 1, :], tmph[:32, 0, :], tmph[:32, 1, :])
                nc.scalar.activation(tc_t[:32, 1, :], cst[:32, 1, :], AF.Tanh)
                nc.vector.tensor_mul(h_sb[:32, 1, :], acts_hi[:32, 1, :], tc_t[:32, 1, :])

                # write this layer's slice into the combined AllGather input
                # layout: [h0_lo(0:128) h1_lo(128:256) h0_hi(256:288) h1_hi(288:320) h2(320:384)]
                nc.sync.dma_start(out=agin[l * 128:(l + 1) * 128, :], in_=h_sb[:, 0, :])
                nc.sync.dma_start(out=agin[256 + 32 * l:256 + 32 * (l + 1), :],
                                  in_=h_sb[:32, 1, :])
                hsb_last[l] = h_sb
                return h_sb

            def lstm_step_l2(t, h1T_t, agin):
                # M-groups of 128: m0 = [i(0:64)|f(64:128)], m1 = [o|g]
                ps = psg2.tile([128, 2, B], f32, name=f"ps2_{t}", tag="ps2")
                for m in range(2):
                    idx = 0
                    for kt in range(NKH2):
                        rhs = ysT[:, kt, (t - 1) * B:t * B] if t > 0 else h2_init[:, kt, :]
                        nc.tensor.matmul(ps[:, m, :], w2h[:, kt, m * 128:(m + 1) * 128],
                                         rhs, start=(idx == 0), stop=False)
                        idx += 1
                    for kt in range(NKH):
                        nc.tensor.matmul(ps[:, m, :], w2i[:, kt, m * 128:(m + 1) * 128],
                                         rhs01(h1T_t, 1, kt), start=False, stop=False)
                        idx += 1
                    nc.tensor.matmul(ps[:, m, :], b2[0:1, m * 128:(m + 1) * 128],
                                     ones[0:1, :], start=False, stop=True)
                # activations: hc2 col0 = sig([i|f]); col1 = [sig(o)(0:64) tanh(g)(64:128)]
                hc2 = wk.tile([128, 2, B], f32, name=f"hc2_{t}", tag="hc2")
                acts = wk.tile([64, 2, B], f32, name=f"acts2_{t}", tag="acts2")
                nc.scalar.activation(hc2[:, 0, :], ps[:, 0, :], AF.Sigmoid)
                nc.scalar.activation(hc2[0:64, 1, :], ps[0:64, 1, :], AF.Sigmoid)
                nc.scalar.activation(hc2[64:128, 1, :], ps[64:128, 1, :], AF.Tanh)
                # realign f and g to base 0 (acts: col0 = f, col1 = tanh(g))
                nc.vector.tensor_copy(acts[:64, 0, :], hc2[64:128, 0, :])
                nc.vector.tensor_copy(acts[:64, 1, :], hc2[64:128, 1, :])
                tmp = wk.tile([64, 2, B], f32, name=f"tmp2_{t}", tag="tmp2")
                tc_t = wk.tile([64, B], f32, name=f"tc2_{t}", tag="tc2")
                h_sb = hop.tile([64, B], bf16, name=f"h2_{t}", tag="h2")
                nc.vector.tensor_mul(tmp[:64, 0, :], acts[:64, 0, :], c2[:64, :])
                nc.vector.tensor_mul(tmp[:64, 1, :], hc2[:64, 0, :], acts[:64, 1, :])
                nc.vector.tensor_add(c2[:64, :], tmp[:64, 0, :], tmp[:64, 1, :])
                nc.scalar.activation(tc_t[:64, :], c2[:64, :], AF.Tanh)
                nc.vector.tensor_mul(h_sb[:64, :], hc2[:64, 1, :], tc_t[:64, :])

                nc.sync.dma_start(out=agin[320:320 + SL2, :], in_=h_sb[:64, :])
                hsb_last[2] = h_sb

            def emit_decode_chunk(c):
                lo = c * TCH * B
                hi = min(TB, (c + 1) * TCH * B)
                ncols = hi - lo
                for m in range((ncols + 127) // 128):
                    mw = min(128, ncols - m * 128)
                    for half in range(2):
                        # stage half a vocab row [2080 cols] in SBUF, 1 DMA out
                        hoff = half * (VS // 2)
                        dst = wk.tile([128, VS // 2], f32, name=f"dst{c}_{m}_{half}",
                                      tag="dst", bufs=2)
                        for off in range(hoff, hoff + VS // 2, 512):
                            nw = min(512, hoff + VS // 2 - off)
                            ps = psd.tile([128, 512], f32, name=f"psd{c}_{m}_{off}",
                                          tag="psd")
                            for kt in range(NKH2):
                                nc.tensor.matmul(
                                    ps[:mw, :nw],
                                    ysT[:, kt, lo + m * 128: lo + m * 128 + mw],
                                    dwt[:, kt, off:off + nw],
                                    start=(kt == 0), stop=(kt == NKH2 - 1))
                            nc.vector.tensor_copy(dst[:mw, off - hoff:off - hoff + nw],
                                                  ps[:mw, :nw])
                        nc.scalar.dma_start(
                            out=dec_d[lo + m * 128: lo + m * 128 + mw,
                                      hoff:hoff + VS // 2],
                            in_=dst[:mw, :])

            # ---- main wavefront loop -------------------------------------
            # combined AllGather block per rank: [h0(160) h1(160) h2(64)] = 384 rows
            AGB = 2 * SL + SL2   # 384
            zero384 = stpool.tile([128, 3, B], bf16, name="zero384")
            nc.gpsimd.memset(zero384[:], 0.0)

            emit_xw0_chunk(0)
            n_slots = t_steps + 2
            for s in range(n_slots):
                # emit xw0 chunk c at slot 8*(c-1)+3 (5 slots before first use)
                if s >= 3 and (s - 3) % TCH == 0:
                    cdue = (s - 3) // TCH + 1
                    if cdue < n_chunks:
                        emit_xw0_chunk(cdue)

                agin = dpool.tile([AGB, B], bf16, name=f"agi_{s}", tag="agi")

                # zero-fill slices of layers not active this slot (pipeline edges)
                def zero_fill(a, b2):
                    while a < b2:
                        n = min(128, b2 - a)
                        nc.sync.dma_start(out=agin[a:a + n, :], in_=zero384[:n, 0, :])
                        a += n
                if not (s < t_steps):
                    zero_fill(0, 128)
                    zero_fill(256, 288)
                if not (0 <= s - 1 < t_steps):
                    zero_fill(128, 256)
                    zero_fill(288, 320)
                if not (0 <= s - 2 < t_steps):
                    zero_fill(320, AGB)

                if s < t_steps:
                    lstm_step_l01(0, s, w0h, w0i, b0, h0T, None, c0, psg0, agin)
                if 0 <= s - 1 < t_steps:
                    t = s - 1
                    lstm_step_l01(1, t, w1h, w1i, b1, h1T, h0T[t + 1], c1, psg1, agin)
                if 0 <= s - 2 < t_steps:
                    t = s - 2
                    lstm_step_l2(t, h1T[t + 1], agin)

                # one AllGather per slot
                agout = dshpool.tile([NCORES * AGB, B], bf16, name=f"ago_{s}",
                                     tag="ago", addr_space="Shared")
                nc.gpsimd.collective_compute(
                    "AllGather", ALU.bypass, ins=[agin.opt()], outs=[agout.opt()],
                    replica_groups=rg)

                # one gather DMA delivers all 16 lo K-tiles in matmul-ready layout:
                # gath[p, k, x, b] = agout[k*384 + x*128 + p, b]
                # (x: 0=h0_lo 1=h1_lo 2=hi-combo [h0hi(32) h1hi(32) h2(64)])
                gath = hxp.tile([128, 8, 3, B], bf16, name=f"gath_{s}", tag="gath")
                nc.sync.dma_start(out=gath[:],
                                  in_=agout.rearrange("(k x p) b -> p k x b", x=3, p=128))
                # view for 32-row leftovers: Aq[r, c, q, b] = agout[(c*4+q)*AGB + r, b]
                Aq = agout.rearrange("(c q r) b -> r c q b", c=2, q=4)
                # view for layer-2 64-row halves: Ah[r, kt, q, b] = agout[(kt*2+q)*AGB + r, b]
                Ah = agout.rearrange("(kt q r) b -> r kt q b", kt=4, q=2)
                if s < t_steps:
                    hi0 = hxp.tile([128, 2, B], bf16, name=f"hi0_{s}", tag="hi0")
                    for q in range(4):
                        nc.scalar.dma_start(out=hi0[32 * q:32 * q + 32, :, :],
                                            in_=Aq[256:288, :, q, :])
                    h0T.append(('g', gath, hi0))
                if 0 <= s - 1 < t_steps:
                    hi1 = hxp.tile([128, 2, B], bf16, name=f"hi1_{s}", tag="hi1")
                    for q in range(4):
                        nc.scalar.dma_start(out=hi1[32 * q:32 * q + 32, :, :],
                                            in_=Aq[288:320, :, q, :])
                    h1T.append(('g', gath, hi1))
                if 0 <= s - 2 < t_steps:
                    t = s - 2
                    for q in range(2):
                        nc.scalar.dma_start(out=ysT[64 * q:64 * q + 64, :, t * B:(t + 1) * B],
                                            in_=Ah[320:384, :, q, :])

                # decode chunk c done when L2 step (c+1)*TCH-1 finished at slot +2
                if s >= 10 and (s - 10) % TCH == 0 and (s - 10) // TCH < n_chunks:
                    emit_decode_chunk((s - 10) // TCH)
            # remaining decode chunks
            first_rem = (n_slots - 10) // TCH + 1 if n_slots >= 10 else 0
            for c in range(max(0, first_rem), n_chunks):
                emit_decode_chunk(c)

            # ---- final states --------------------------------------------
            for l, (hd, cd, cst) in enumerate([(hT0_d, cT0_d, c0), (hT1_d, cT1_d, c1)]):
                f = wk.tile([128, 2, B], f32, name=f"fin{l}", tag="fin", bufs=2)
                nc.gpsimd.memset(f[:], 0.0)
                nc.vector.tensor_copy(f[:, 0, :], hsb_last[l][:, 0, :])
                nc.vector.tensor_copy(f[:32, 1, :], hsb_last[l][:32, 1, :])
                nc.sync.dma_start(out=hd[:], in_=f[:])
                nc.sync.dma_start(out=cd[:], in_=cst[:])
            f2 = wk.tile([128, B], f32, name="fin2", tag="fin2")
            nc.gpsimd.memset(f2[:], 0.0)
            nc.vector.tensor_copy(f2[:64, :], hsb_last[2][:64, :])
            nc.sync.dma_start(out=hT2_d[:], in_=f2[:])
            nc.sync.dma_start(out=cT2_d[:], in_=c2[:])

    nc.compile()
    return nc


# ----------------------------------------------------------------------------
# entry point
# ----------------------------------------------------------------------------

_cached = {}


def _get_program(t_steps=T):
    if t_steps not in _cached:
        _cached[t_steps] = build_program(t_steps)
    return _cached[t_steps]


def run(inputs, t_steps=T, trace=False, trace_kwargs=None):
    from concourse.bass_utils import run_bass_kernel_spmd
    nc = _get_program(t_steps)
    in_maps = prepare_inputs(inputs, t_steps)
    res = run_bass_kernel_spmd(nc, in_maps, core_ids=list(range(NCORES)),
                               trace=trace, **(trace_kwargs or {}))
    return res


def assemble_outputs(results, inputs, t_steps=T):
    TB_ = t_steps * B
    decoded = np.empty((t_steps, B, V), np.float32)
    for k in range(NCORES):
        d = results[k]['dec'].reshape(t_steps, B, VS)
        vlo, vhi = VS * k, min(V, VS * (k + 1))
        decoded[:, :, vlo:vhi] = d[:, :, :vhi - vlo]
    dec_b = np.asarray(inputs['dec_b'], np.float32)
    if np.any(dec_b):
        decoded += dec_b

    def unpack_l01(key, hsz):
        out = np.zeros((B, hsz), np.float32)
        for k in range(NCORES):
            a = results[k][key]          # [128, 2, B]
            sl = np.concatenate([a[:, 0, :], a[:32, 1, :]], axis=0)  # [160, B]
            for i, j in enumerate(_slice_rows_l01(k)):
                if j < hsz:
                    out[:, j] = sl[i]
        return out

    def unpack_l2(key, hsz):
        out = np.zeros((B, hsz), np.float32)
        for k in range(NCORES):
            a = results[k][key]          # [128, B]
            lo, hi = SL2 * k, min(hsz, SL2 * (k + 1))
            if hi > lo:
                out[:, lo:hi] = a[:hi - lo].T
        return out

    hT0 = unpack_l01('hT0', H); cT0 = unpack_l01('cT0', H)
    hT1 = unpack_l01('hT1', H); cT1 = unpack_l01('cT1', H)
    hT2 = unpack_l2('hT2', E); cT2 = unpack_l2('cT2', E)
    return (decoded, hT0, cT0, hT1, cT1, hT2, cT2)


def kernel(**inputs):
    res = run(inputs, t_steps=T)
    return assemble_outputs(res.results, inputs, t_steps=T)


# revision 69
# speedup vs baseline: 1.0902x; 1.0371x over previous
"""AWD-LSTM forward on 8 Trainium2 NeuronCores (Bass/Tile, SPMD).

Strategy:
  - 8-way tensor parallelism over the hidden/gate dimension for all three LSTM
    layers (H=1150 -> padded 1280, 160 rows/core; layer2 H=400 -> 512, 64/core),
    with a per-step AllGather of h (bf16).
  - Everything lives in transposed layout: features on partitions, batch (80)
    on the free dim.  Per-step gates G.T[:, b] accumulate in PSUM from
    (a) Whh_slice.T.T @ h.T (recurrent) and, for layers 1/2, (b) Wih_slice @ x.T
    fused into the same accumulation group.  Layer0's input contribution is a
    bulk GEMM from the (host-gathered) embedding, staged through DRAM.
  - Vocab projection is sharded over V (33280/8 = 4160 cols/core), computed in
    time-chunks from the accumulated ys.T history, PSUM DMA'd straight to HBM.
  - Emission follows the wavefront (slot s: L0 step s, L1 step s-1, L2 step
    s-2, due xw0/decode chunks) so each engine's in-order stream interleaves
    layers and the AllGather latency is hidden by compute of other layers.
  - bf16 matmuls, fp32 cell state & PSUM accumulation.
"""
import os
import sys

sys.path.insert(0, '/opt/trn_rl_repo')

import numpy as np
import ml_dtypes

BF16 = ml_dtypes.bfloat16

# problem shapes (hardcoded per contract)
V, E, H, T, B = 33278, 400, 1150, 70, 80
NCORES = 8
EP = 512            # padded E (K dim)
HP = 1280           # padded H (layers 0,1)
SL = HP // NCORES   # 160 hidden rows per core
H2P = 512           # padded layer-2 H (=400)
SL2 = H2P // NCORES  # 64
VP = 33280
VS = VP // NCORES   # 4160
NKE = EP // 128     # 4   K-tiles over embedding dim
NKH = HP // 128     # 10  K-tiles over H (layers 0,1)
NKH2 = H2P // 128   # 4   K-tiles over layer-2 H
NM01 = 5            # M-tiles of per-core gates, layers 0/1 (640 rows)
NM2 = 2             # M-tiles of per-core gates, layer 2 (256 rows)
TCH = 8             # decode/xw0 time-chunk (steps)


# ----------------------------------------------------------------------------
# host-side prep
# ----------------------------------------------------------------------------

def _slice_rows_l01(k):
    """Core k's hidden rows (padded H=1280): a 128-block plus a 32-block.

    This split makes the combined-AllGather output (per-rank blocks of
    [h0(128+32) h1(128+32) h2(64)]) map back onto plain contiguous 128-row
    K-tiles with affine DMA access patterns.
    """
    return list(range(128 * k, 128 * (k + 1))) + \
        list(range(1024 + 32 * k, 1024 + 32 * (k + 1)))


def _gate_row_map_l01(k):
    # M order: [i_lo(128) f_lo(128) o_lo(128) g_lo(128) | i_hi f_hi o_hi g_hi (32 each)]
    # orig (PyTorch) gate order: i=0 f=1 g=2 o=3;  ours [i,f,o,g] -> [0,1,3,2]
    order = [0, 1, 3, 2]
    sr = _slice_rows_l01(k)
    rows = []
    for m in range(640):
        if m < 512:
            gate = order[m // 128]
            j = sr[m % 128]
        else:
            sub = m - 512
            gate = order[sub // 32]
            j = sr[128 + (sub % 32)]
        rows.append((gate, j))
    return rows


def _gate_row_map_l2(k):
    order = [0, 1, 3, 2]
    return [(order[m // 64], SL2 * k + (m % 64)) for m in range(256)]


def _make_wT(Wih, Whh, bih, bhh, k, hsz, row_map, in_sz, in_pad, hpad):
    nM = len(row_map)
    wi = np.zeros((in_pad, nM), np.float32)
    wh = np.zeros((hpad, nM), np.float32)
    b = np.zeros(nM, np.float32)
    for m, (gate, j) in enumerate(row_map):
        if j < hsz:
            r = gate * hsz + j
            wi[:in_sz, m] = Wih[r]
            wh[:hsz, m] = Whh[r]
            b[m] = bih[r] + bhh[r]
    return wi.astype(BF16), wh.astype(BF16), b


def _fold_k(a, nk):
    """[nk*128, N] -> [128, nk, N] with row r=(kt*128+p) -> [p, kt]."""
    n = a.shape[1]
    return np.ascontiguousarray(a.reshape(nk, 128, n).transpose(1, 0, 2))


def _bias_tiles_l01(b):
    # b: [640] (4x128 lo + 4x32 hi) -> [128, 5]: cols 0:4 lo gates, col 4 = hi combo
    out = np.zeros((128, 5), np.float32)
    for g in range(4):
        out[:, g] = b[g * 128:(g + 1) * 128]
        out[32 * g:32 * (g + 1), 4] = b[512 + g * 32: 512 + (g + 1) * 32]
    return out


def _bias_tiles_l2(b):
    # b: [256] (4x64, order i f o g) -> [128, 2]: col0 = [i|f], col1 = [o|g]
    out = np.zeros((128, 2), np.float32)
    out[:64, 0] = b[0:64]
    out[64:, 0] = b[64:128]
    out[:64, 1] = b[128:192]
    out[64:, 1] = b[192:256]
    return out


def _full_hT(h0, hsz, hpad, nk):
    out = np.zeros((hpad, B), np.float32)
    out[:hsz] = np.asarray(h0, np.float32).T
    return _fold_k(out.astype(BF16), nk)


def _c_pack_l01(c0, k):
    # per-core c slice -> [128, 2, 80] (col0 = 128-block, col1[:32] = 32-block)
    out = np.zeros((128, 2, B), np.float32)
    c0 = np.asarray(c0, np.float32)
    sr = _slice_rows_l01(k)
    sl = np.zeros((SL, B), np.float32)
    for i, j in enumerate(sr):
        if j < H:
            sl[i] = c0[:, j]
    out[:, 0, :] = sl[:128]
    out[:32, 1, :] = sl[128:160]
    return out


def _c_pack_l2(c0, k):
    out = np.zeros((128, B), np.float32)
    lo, hi = SL2 * k, min(E, SL2 * (k + 1))
    if hi > lo:
        out[:hi - lo] = np.asarray(c0, np.float32)[:, lo:hi].T
    return out


def prepare_inputs(inputs, t_steps=T):
    """Returns in_maps: list of 8 dicts keyed by DRAM tensor name."""
    tokens = np.asarray(inputs['tokens'])[:t_steps]
    emb_W = np.asarray(inputs['emb_W'], np.float32)
    dec_W = np.asarray(inputs['dec_W'], np.float32)

    x = emb_W[tokens.reshape(-1)]                    # [T*B, 400]
    xT = np.zeros((EP, t_steps * B), np.float32)
    xT[:E] = x.T
    xT_f = _fold_k(xT.astype(BF16), NKE)             # [128, 4, 5600]

    h0i = _full_hT(inputs['h0_0'], H, HP, NKH)
    h1i = _full_hT(inputs['h0_1'], H, HP, NKH)
    h2i = _full_hT(inputs['h0_2'], E, H2P, NKH2)

    Ws = []
    for l, (in_sz, hsz) in enumerate([(E, H), (H, H), (H, E)]):
        Ws.append((np.asarray(inputs[f'Wih{l}'], np.float32),
                   np.asarray(inputs[f'Whh{l}'], np.float32),
                   np.asarray(inputs[f'bih{l}'], np.float32),
                   np.asarray(inputs[f'bhh{l}'], np.float32)))

    in_maps = []
    for k in range(NCORES):
        rm0 = _gate_row_map_l01(k)
        w0i, w0h, b0 = _make_wT(Ws[0][0], Ws[0][1], Ws[0][2], Ws[0][3], k, H, rm0, E, EP, HP)
        w1i, w1h, b1 = _make_wT(Ws[1][0], Ws[1][1], Ws[1][2], Ws[1][3], k, H, rm0, H, HP, HP)
        rm2 = _gate_row_map_l2(k)
        w2i, w2h, b2 = _make_wT(Ws[2][0], Ws[2][1], Ws[2][2], Ws[2][3], k, E, rm2, H, HP, H2P)

        dwt = np.zeros((H2P, VS), np.float32)
        vlo, vhi = VS * k, min(V, VS * (k + 1))
        dwt[:E, :vhi - vlo] = dec_W[vlo:vhi].T

        m = {
            'xT': xT_f,
            'w0i': _fold_k(w0i, NKE), 'w0h': _fold_k(w0h, NKH), 'b0': _bias_tiles_l01(b0),
            'w1i': _fold_k(w1i, NKH), 'w1h': _fold_k(w1h, NKH),
            'b1': b1.reshape(1, 640).astype(BF16),
            'w2i': _fold_k(w2i, NKH), 'w2h': _fold_k(w2h, NKH2),
            'b2': b2.reshape(1, 256).astype(BF16),
            'dwt': _fold_k(dwt.astype(BF16), NKH2),
            'h0i': h0i, 'h1i': h1i, 'h2i': h2i,
            'c0i': _c_pack_l01(inputs['c0_0'], k),
            'c1i': _c_pack_l01(inputs['c0_1'], k),
            'c2i': _c_pack_l2(inputs['c0_2'], k),
        }
        in_maps.append(m)
    return in_maps


# ----------------------------------------------------------------------------
# device program
# ----------------------------------------------------------------------------

def build_program(t_steps=T, n_cores=NCORES):
    import concourse.bass as bass
    import concourse.bacc as bacc
    import concourse.mybir as mybir
    import concourse.tile as tile

    f32 = mybir.dt.float32
    bf16 = mybir.dt.bfloat16
    AF = mybir.ActivationFunctionType
    ALU = mybir.AluOpType

    TB = t_steps * B
    n_chunks = (t_steps + TCH - 1) // TCH

    nc = bacc.Bacc("TRN2", target_bir_lowering=False, debug=False,
                   num_devices=n_cores)

    # ---- I/O -------------------------------------------------------------
    xT_d = nc.dram_tensor('xT', [128, NKE, TB], bf16, kind="ExternalInput")
    w0i_d = nc.dram_tensor('w0i', [128, NKE, 640], bf16, kind="ExternalInput")
    w0h_d = nc.dram_tensor('w0h', [128, NKH, 640], bf16, kind="ExternalInput")
    b0_d = nc.dram_tensor('b0', [128, 5], f32, kind="ExternalInput")
    w1i_d = nc.dram_tensor('w1i', [128, NKH, 640], bf16, kind="ExternalInput")
    w1h_d = nc.dram_tensor('w1h', [128, NKH, 640], bf16, kind="ExternalInput")
    b1_d = nc.dram_tensor('b1', [1, 640], bf16, kind="ExternalInput")
    w2i_d = nc.dram_tensor('w2i', [128, NKH, 256], bf16, kind="ExternalInput")
    w2h_d = nc.dram_tensor('w2h', [128, NKH2, 256], bf16, kind="ExternalInput")
    b2_d = nc.dram_tensor('b2', [1, 256], bf16, kind="ExternalInput")
    dwt_d = nc.dram_tensor('dwt', [128, NKH2, VS], bf16, kind="ExternalInput")
    h0i_d = nc.dram_tensor('h0i', [128, NKH, B], bf16, kind="ExternalInput")
    h1i_d = nc.dram_tensor('h1i', [128, NKH, B], bf16, kind="ExternalInput")
    h2i_d = nc.dram_tensor('h2i', [128, NKH2, B], bf16, kind="ExternalInput")
    c0i_d = nc.dram_tensor('c0i', [128, 2, B], f32, kind="ExternalInput")
    c1i_d = nc.dram_tensor('c1i', [128, 2, B], f32, kind="ExternalInput")
    c2i_d = nc.dram_tensor('c2i', [128, B], f32, kind="ExternalInput")

    dec_d = nc.dram_tensor('dec', [TB, VS], f32, kind="ExternalOutput")
    hT0_d = nc.dram_tensor('hT0', [128, 2, B], f32, kind="ExternalOutput")
    cT0_d = nc.dram_tensor('cT0', [128, 2, B], f32, kind="ExternalOutput")
    hT1_d = nc.dram_tensor('hT1', [128, 2, B], f32, kind="ExternalOutput")
    cT1_d = nc.dram_tensor('cT1', [128, 2, B], f32, kind="ExternalOutput")
    hT2_d = nc.dram_tensor('hT2', [128, B], f32, kind="ExternalOutput")
    cT2_d = nc.dram_tensor('cT2', [128, B], f32, kind="ExternalOutput")

    # internal DRAM: layer0 input contributions.
    # M-groups: cols 0:4 = lo gates (128 rows), col 4 = hi combo [i f o g]x32.
    xw0_d = nc.dram_tensor('xw0', [128, 5, TB], f32)

    rg = [list(range(n_cores))]

    with tile.TileContext(nc) as tc:
        with (
            tc.tile_pool(name="wpool", bufs=1) as wpool,
            tc.tile_pool(name="ys", bufs=1) as yspool,
            tc.tile_pool(name="state", bufs=1) as stpool,
            tc.tile_pool(name="xtp", bufs=2) as xtp,
            tc.tile_pool(name="hx", bufs=3) as hxp,
            tc.tile_pool(name="work", bufs=2) as wk,
            tc.tile_pool(name="hout", bufs=3) as hop,
            tc.tile_pool(name="xw0sb", bufs=3) as xw0p,
            tc.tile_pool(name="psg0", bufs=1, space="PSUM") as psg0,
            tc.tile_pool(name="psg1", bufs=1, space="PSUM") as psg1,
            tc.tile_pool(name="psg2", bufs=1, space="PSUM") as psg2,
            tc.tile_pool(name="psx", bufs=1, space="PSUM") as psx,
            tc.tile_pool(name="psd", bufs=2, space="PSUM") as psd,
            tc.tile_pool(name="dram", bufs=3, space="DRAM") as dpool,
            tc.tile_pool(name="dramsh", bufs=3, space="DRAM") as dshpool,
        ):
            # ---- resident weights ----------------------------------------
            w0i = wpool.tile([128, NKE, 640], bf16)
            w0h = wpool.tile([128, NKH, 640], bf16)
            w1i = wpool.tile([128, NKH, 640], bf16)
            w1h = wpool.tile([128, NKH, 640], bf16)
            w2i = wpool.tile([128, NKH, 256], bf16)
            w2h = wpool.tile([128, NKH2, 256], bf16)
            dwt = wpool.tile([128, NKH2, VS], bf16)
            b0 = wpool.tile([128, 5], f32)
            b1 = wpool.tile([1, 640], bf16, padded_shape=[128, 640])
            b2 = wpool.tile([1, 256], bf16, padded_shape=[128, 256])
            ones = wpool.tile([1, B], bf16, padded_shape=[128, B])
            nc.gpsimd.memset(ones[0:1, :], 1.0)
            for sb, dr in [(w0i, w0i_d), (w0h, w0h_d), (w1i, w1i_d), (w1h, w1h_d),
                           (w2i, w2i_d), (w2h, w2h_d), (dwt, dwt_d),
                           (b0, b0_d), (b1, b1_d), (b2, b2_d)]:
                nc.scalar.dma_start(out=sb[:], in_=dr[:])

            # ys history (layer-2 h over all time) — decode lhsT
            ysT = yspool.tile([128, NKH2, TB], bf16)

            # persistent cell state
            c0 = stpool.tile([128, 2, B], f32)
            c1 = stpool.tile([128, 2, B], f32)
            c2 = stpool.tile([128, B], f32)
            nc.sync.dma_start(out=c0[:], in_=c0i_d[:])
            nc.sync.dma_start(out=c1[:], in_=c1i_d[:])
            nc.sync.dma_start(out=c2[:], in_=c2i_d[:])

            # initial h (full, replicated)
            h0_init = stpool.tile([128, NKH, B], bf16)
            h1_init = stpool.tile([128, NKH, B], bf16)
            h2_init = stpool.tile([128, NKH2, B], bf16)
            nc.sync.dma_start(out=h0_init[:], in_=h0i_d[:])
            nc.sync.dma_start(out=h1_init[:], in_=h1i_d[:])
            nc.sync.dma_start(out=h2_init[:], in_=h2i_d[:])

            # rolling full-h entries (from AllGather).  Entry forms:
            #   ('full', tile[128, NKH, B])            -- initial state
            #   ('g', gath[128, 8, 3, B], hi[128, 2, B]) -- gathered slot
            h0T = [('full', h0_init)]   # h0T[t] = h_{t-1} for step t
            h1T = [('full', h1_init)]
            hsb_last = {}

            def rhs01(entry, x, kt):
                """K-tile kt of a layer-0/1 hidden state."""
                if entry[0] == 'full':
                    return entry[1][:, kt, :]
                if kt < 8:
                    return entry[1][:, kt, 0, :]
                return entry[2][:, kt - 8, :]

            # ---- helpers --------------------------------------------------
            def emit_xw0_chunk(c):
                """xw0[:, :, cols] = w0i.T @ xT[:, :, cols] for time-chunk c."""
                lo = c * TCH * B
                hi = min(TB, (c + 1) * TCH * B)
                ncols = hi - lo
                xt = xtp.tile([128, NKE, ncols], bf16, name=f"xt{c}",
                              tag="xt", bufs=1, padded_shape=[128, NKE, TCH * B])
                nc.scalar.dma_start(out=xt[:], in_=xT_d[:, :, lo:hi])
                for m in range(5):
                    mw, mc = 128, m * 128
                    for off in range(0, ncols, 512):
                        nw = min(512, ncols - off)
                        ps = psx.tile([128, 512], f32, name=f"psx{c}_{m}_{off}", tag="psx")
                        for kt in range(NKE):
                            nc.tensor.matmul(ps[:mw, :nw],
                                             w0i[:, kt, mc:mc + mw],
                                             xt[:, kt, off:off + nw],
                                             start=(kt == 0), stop=(kt == NKE - 1))
                        xsb = xtp.tile([128, 512], f32, name=f"xsb{c}_{m}_{off}",
                                       tag="xsb", bufs=2)
                        nc.vector.tensor_copy(xsb[:mw, :nw], ps[:mw, :nw])
                        nc.scalar.dma_start(out=xw0_d[:mw, m, lo + off:lo + off + nw],
                                            in_=xsb[:mw, :nw])

            def lstm_step_l01(l, t, wh, wi, bias, hT_list, other_hT, cst, psg, agin):
                """One step of layer 0 or 1.

                M-groups are gate-pure: ps_lo[:, g, :] holds gate g rows 0:128
                of this core's slice; ps_hi[:32, g, :] holds rows 128:160.
                Gate order g: 0=i 1=f 2=o 3=g.
                """
                ps_lo = psg.tile([128, 4, B], f32, name=f"pslo{l}_{t}", tag=f"pslo{l}")
                ps_hi = psg.tile([128, B], f32, name=f"pshi{l}_{t}", tag=f"pshi{l}")
                hprev = hT_list[t]  # entry; hT_list[t] = h_{t-1}
                nmm = NKH + (NKH + 1 if l == 1 else 0)
                for g in range(5):
                    # g<4: lo gate groups; g==4: hi combo [i f o g]x32
                    ps = ps_lo[:, g, :] if g < 4 else ps_hi[:, :]
                    mc = g * 128
                    idx = 0
                    for kt in range(NKH):
                        nc.tensor.matmul(ps, wh[:, kt, mc:mc + 128],
                                         rhs01(hprev, l, kt),
                                         start=(idx == 0), stop=(idx == nmm - 1))
                        idx += 1
                    if l == 1:
                        for kt in range(NKH):
                            nc.tensor.matmul(ps, wi[:, kt, mc:mc + 128],
                                             rhs01(other_hT, 0, kt),
                                             start=False, stop=False)
                            idx += 1
                        # bias via K=1 matmul: bias_row.T @ ones
                        nc.tensor.matmul(ps, bias[0:1, mc:mc + 128], ones[0:1, :],
                                         start=False, stop=True)

                acts_lo = wk.tile([128, 4, B], f32, name=f"aclo{l}_{t}", tag=f"aclo{l}")
                # hi combo activations (gate-major [i f o g]x32), then DVE copies
                # realign f/o/g to partition base 0
                hc = wk.tile([128, B], f32, name=f"hc{l}_{t}", tag=f"hc{l}")
                acts_hi = wk.tile([32, 3, B], f32, name=f"achi{l}_{t}", tag=f"achi{l}")
                if l == 0:
                    # gates = psum + bias + xw0_t ; then activations
                    xw = xw0p.tile([128, 5, B], f32, name=f"xw0t{t}", tag="xw0t")
                    nc.scalar.dma_start(out=xw[:], in_=xw0_d[:, :, t * B:(t + 1) * B])
                    glo = wk.tile([128, 4, B], f32, name=f"glo{l}_{t}", tag=f"glo{l}")
                    ghi = wk.tile([128, B], f32, name=f"ghi{l}_{t}", tag=f"ghi{l}")
                    for g in range(4):
                        nc.vector.scalar_tensor_tensor(
                            glo[:, g, :], ps_lo[:, g, :], bias[:, g:g + 1],
                            xw[:, g, :], op0=ALU.add, op1=ALU.add)
                    nc.vector.scalar_tensor_tensor(
                        ghi[:, :], ps_hi[:, :], bias[:, 4:5],
                        xw[:, 4, :], op0=ALU.add, op1=ALU.add)
                    nc.scalar.activation(acts_lo[:, 0:3, :], glo[:, 0:3, :], AF.Sigmoid)
                    nc.scalar.activation(acts_lo[:, 3, :], glo[:, 3, :], AF.Tanh)
                    nc.scalar.activation(hc[0:96, :], ghi[0:96, :], AF.Sigmoid)
                    nc.scalar.activation(hc[96:128, :], ghi[96:128, :], AF.Tanh)
                else:
                    # bias already accumulated in PSUM (K=1 matmul)
                    nc.scalar.activation(acts_lo[:, 0:3, :], ps_lo[:, 0:3, :], AF.Sigmoid)
                    nc.scalar.activation(acts_lo[:, 3, :], ps_lo[:, 3, :], AF.Tanh)
                    nc.scalar.activation(hc[0:96, :], ps_hi[0:96, :], AF.Sigmoid)
                    nc.scalar.activation(hc[96:128, :], ps_hi[96:128, :], AF.Tanh)
                # realign hi f/o/g to base 0 (cols 0=f 1=o 2=g); i read from hc[0:32]
                nc.vector.tensor_copy(acts_hi[:32, 0, :], hc[32:64, :])
                nc.vector.tensor_copy(acts_hi[:32, 1, :], hc[64:96, :])
                nc.vector.tensor_copy(acts_hi[:32, 2, :], hc[96:128, :])

                # c update: c = sig(f)*c + sig(i)*tanh(g); h = sig(o)*tanh(c)
                tmp = wk.tile([128, 2, B], f32, name=f"tmp{l}_{t}", tag=f"tmp{l}")
                tmph = wk.tile([32, 2, B], f32, name=f"tmph{l}_{t}", tag=f"tmph{l}")
                tc_t = wk.tile([128, 2, B], f32, name=f"tc{l}_{t}", tag=f"tc{l}")
                h_sb = hop.tile([128, 2, B], bf16, name=f"h{l}_{t}", tag=f"h{l}")
                # lo
                nc.vector.tensor_mul(tmp[:, 0, :], acts_lo[:, 1, :], cst[:, 0, :])
                nc.vector.tensor_mul(tmp[:, 1, :], acts_lo[:, 0, :], acts_lo[:, 3, :])
                nc.vector.tensor_add(cst[:, 0, :], tmp[:, 0, :], tmp[:, 1, :])
                nc.scalar.activation(tc_t[:, 0, :], cst[:, 0, :], AF.Tanh)
                nc.vector.tensor_mul(h_sb[:, 0, :], acts_lo[:, 2, :], tc_t[:, 0, :])
                # hi (32 rows, all at partition base 0)
                nc.vector.tensor_mul(tmph[:32, 0, :], acts_hi[:32, 0, :], cst[:32, 1, :])
                nc.vector.tensor_mul(tmph[:32, 1, :], hc[:32, :], acts_hi[:32, 2, :])
                nc.vector.tensor_add(cst[:32, 1, :], tmph[:32, 0, :], tmph[:32, 1, :])
                nc.scalar.activation(tc_t[:32, 1, :], cst[:32, 1, :], AF.Tanh)
                nc.vector.tensor_mul(h_sb[:32, 1, :], acts_hi[:32, 1, :], tc_t[:32, 1, :])

                # write this layer's slice into its AllGather input block
                # block layout: [h_lo(0:128) | h_hi(128:160) | (h2 160:224) | pad]
                nc.sync.dma_start(out=agin[0:128, :], in_=h_sb[:, 0, :])
                nc.sync.dma_start(out=agin[128:160, :], in_=h_sb[:32, 1, :])
                hsb_last[l] = h_sb
                return h_sb

            def lstm_step_l2(t, h1T_t, agin):
                # M-groups of 128: m0 = [i(0:64)|f(64:128)], m1 = [o|g]
                ps = psg2.tile([128, 2, B], f32, name=f"ps2_{t}", tag="ps2")
                for m in range(2):
                    idx = 0
                    for kt in range(NKH2):
                        rhs = ysT[:, kt, (t - 1) * B:t * B] if t > 0 else h2_init[:, kt, :]
                        nc.tensor.matmul(ps[:, m, :], w2h[:, kt, m * 128:(m + 1) * 128],
                                         rhs, start=(idx == 0), stop=False)
                        idx += 1
                    for kt in range(NKH):
                        nc.tensor.matmul(ps[:, m, :], w2i[:, kt, m * 128:(m + 1) * 128],
                                         rhs01(h1T_t, 1, kt), start=False, stop=False)
                        idx += 1
                    nc.tensor.matmul(ps[:, m, :], b2[0:1, m * 128:(m + 1) * 128],
                                     ones[0:1, :], start=False, stop=True)
                # activations: hc2 col0 = sig([i|f]); col1 = [sig(o)(0:64) tanh(g)(64:128)]
                hc2 = wk.tile([128, 2, B], f32, name=f"hc2_{t}", tag="hc2")
                acts = wk.tile([64, 2, B], f32, name=f"acts2_{t}", tag="acts2")
                nc.scalar.activation(hc2[:, 0, :], ps[:, 0, :], AF.Sigmoid)
                nc.scalar.activation(hc2[0:64, 1, :], ps[0:64, 1, :], AF.Sigmoid)
                nc.scalar.activation(hc2[64:128, 1, :], ps[64:128, 1, :], AF.Tanh)
                # realign f and g to base 0 (acts: col0 = f, col1 = tanh(g))
                nc.vector.tensor_copy(acts[:64, 0, :], hc2[64:128, 0, :])
                nc.vector.tensor_copy(acts[:64, 1, :], hc2[64:128, 1, :])
                tmp = wk.tile([64, 2, B], f32, name=f"tmp2_{t}", tag="tmp2")
                tc_t = wk.tile([64, B], f32, name=f"tc2_{t}", tag="tc2")
                h_sb = hop.tile([64, B], bf16, name=f"h2_{t}", tag="h2")
                nc.vector.tensor_mul(tmp[:64, 0, :], acts[:64, 0, :], c2[:64, :])
                nc.vector.tensor_mul(tmp[:64, 1, :], hc2[:64, 0, :], acts[:64, 1, :])
                nc.vector.tensor_add(c2[:64, :], tmp[:64, 0, :], tmp[:64, 1, :])
                nc.scalar.activation(tc_t[:64, :], c2[:64, :], AF.Tanh)
                nc.vector.tensor_mul(h_sb[:64, :], hc2[:64, 1, :], tc_t[:64, :])

                nc.sync.dma_start(out=agin[160:160 + SL2, :], in_=h_sb[:64, :])
                hsb_last[2] = h_sb

            def emit_decode_mgroup(c, m):
                lo = c * TCH * B
                ncols = min(TB, (c + 1) * TCH * B) - lo
                mw = min(128, ncols - m * 128)
                for half in range(2):
                    # stage half a vocab row [2080 cols] in SBUF, 1 DMA out
                    hoff = half * (VS // 2)
                    dst = wk.tile([128, VS // 2], f32, name=f"dst{c}_{m}_{half}",
                                  tag="dst", bufs=2)
                    for off in range(hoff, hoff + VS // 2, 512):
                        nw = min(512, hoff + VS // 2 - off)
                        ps = psd.tile([128, 512], f32, name=f"psd{c}_{m}_{off}",
                                      tag="psd")
                        for kt in range(NKH2):
                            nc.tensor.matmul(
                                ps[:mw, :nw],
                                ysT[:, kt, lo + m * 128: lo + m * 128 + mw],
                                dwt[:, kt, off:off + nw],
                                start=(kt == 0), stop=(kt == NKH2 - 1))
                        nc.vector.tensor_copy(dst[:mw, off - hoff:off - hoff + nw],
                                              ps[:mw, :nw])
                    nc.scalar.dma_start(
                        out=dec_d[lo + m * 128: lo + m * 128 + mw,
                                  hoff:hoff + VS // 2],
                        in_=dst[:mw, :])

            # ---- main wavefront loop -------------------------------------
            # Two AllGathers per slot, each 256 rows/rank:
            #   A: [h0_lo(128) | h0_hi(32) pad(96)]        (fires after L0)
            #   B: [h1_lo(128) | h1_hi(32) h2(64) pad(32)] (fires after L1+L2)
            zero384 = stpool.tile([128, 3, B], bf16, name="zero384")
            nc.gpsimd.memset(zero384[:], 0.0)
            pending_dec = []
            chunks_enqueued = set()

            def emit_ag(agin, tag, s):
                agout = dshpool.tile([NCORES * 256, B], bf16, name=f"ago{tag}_{s}",
                                     tag=f"ago{tag}", addr_space="Shared")
                nc.gpsimd.collective_compute(
                    "AllGather", ALU.bypass, ins=[agin.opt()], outs=[agout.opt()],
                    replica_groups=rg)
                gath = hxp.tile([128, 8, 2, B], bf16, name=f"gath{tag}_{s}",
                                tag=f"gath{tag}", bufs=2)
                nc.sync.dma_start(out=gath[:],
                                  in_=agout.rearrange("(k x p) b -> p k x b", x=2, p=128))
                return agout, gath

            def zero_fill(agin, a, b2):
                while a < b2:
                    n = min(128, b2 - a)
                    nc.sync.dma_start(out=agin[a:a + n, :], in_=zero384[:n, 0, :])
                    a += n

            emit_xw0_chunk(0)
            n_slots = t_steps + 2
            for s in range(n_slots):
                # emit xw0 chunk c at slot 8*(c-1)+3 (5 slots before first use)
                if s >= 3 and (s - 3) % TCH == 0:
                    cdue = (s - 3) // TCH + 1
                    if cdue < n_chunks:
                        emit_xw0_chunk(cdue)
                if s >= 10 and (s - 10) % TCH == 0 and (s - 10) // TCH < n_chunks:
                    c = (s - 10) // TCH
                    ncols = min(TB, (c + 1) * TCH * B) - c * TCH * B
                    pending_dec.extend((c, m) for m in range((ncols + 127) // 128))
                    chunks_enqueued.add(c)

                # ---- layer 0 + AllGather A --------------------------------
                if s < t_steps:
                    aginA = dpool.tile([256, B], bf16, name=f"agiA_{s}", tag="agiA")
                    lstm_step_l01(0, s, w0h, w0i, b0, h0T, None, c0, psg0, aginA)
                    agoutA, gathA = emit_ag(aginA, "A", s)
                    AqA = agoutA.rearrange("(c q r) b -> r c q b", c=2, q=4)
                    hi0 = hxp.tile([128, 2, B], bf16, name=f"hi0_{s}", tag="hi0")
                    for q in range(4):
                        nc.scalar.dma_start(out=hi0[32 * q:32 * q + 32, :, :],
                                            in_=AqA[128:160, :, q, :])
                    h0T.append(('g', gathA, hi0))

                # decode fill (overlaps AllGather A latency)
                if pending_dec:
                    emit_decode_mgroup(*pending_dec.pop(0))

                # ---- layers 1,2 + AllGather B -----------------------------
                if 0 <= s - 1 < t_steps or 0 <= s - 2 < t_steps:
                    aginB = dpool.tile([256, B], bf16, name=f"agiB_{s}", tag="agiB")
                    if 0 <= s - 1 < t_steps:
                        t = s - 1
                        lstm_step_l01(1, t, w1h, w1i, b1, h1T, h0T[t + 1], c1, psg1,
                                      aginB)
                    else:
                        zero_fill(aginB, 0, 160)
                    if 0 <= s - 2 < t_steps:
                        t = s - 2
                        lstm_step_l2(t, h1T[t + 1], aginB)
                    else:
                        zero_fill(aginB, 160, 224)
                    agoutB, gathB = emit_ag(aginB, "B", s)
                    AqB = agoutB.rearrange("(c q r) b -> r c q b", c=2, q=4)
                    AhB = agoutB.rearrange("(kt q r) b -> r kt q b", kt=4, q=2)
                    if 0 <= s - 1 < t_steps:
                        hi1 = hxp.tile([128, 2, B], bf16, name=f"hi1_{s}", tag="hi1")
                        for q in range(4):
                            nc.scalar.dma_start(out=hi1[32 * q:32 * q + 32, :, :],
                                                in_=AqB[128:160, :, q, :])
                        h1T.append(('g', gathB, hi1))
                    if 0 <= s - 2 < t_steps:
                        t = s - 2
                        for q in range(2):
                            nc.scalar.dma_start(
                                out=ysT[64 * q:64 * q + 64, :, t * B:(t + 1) * B],
                                in_=AhB[160:224, :, q, :])
            # remaining decode m-groups (incl. chunks never enqueued in-loop)
            for c in range(n_chunks):
                if c not in chunks_enqueued:
                    ncols = min(TB, (c + 1) * TCH * B) - c * TCH * B
                    pending_dec.extend((c, m) for m in range((ncols + 127) // 128))
            for c, m in pending_dec:
                emit_decode_mgroup(c, m)

            # ---- final states --------------------------------------------
            for l, (hd, cd, cst) in enumerate([(hT0_d, cT0_d, c0), (hT1_d, cT1_d, c1)]):
                f = wk.tile([128, 2, B], f32, name=f"fin{l}", tag="fin", bufs=2)
                nc.gpsimd.memset(f[:], 0.0)
                nc.vector.tensor_copy(f[:, 0, :], hsb_last[l][:, 0, :])
                nc.vector.tensor_copy(f[:32, 1, :], hsb_last[l][:32, 1, :])
                nc.sync.dma_start(out=hd[:], in_=f[:])
                nc.sync.dma_start(out=cd[:], in_=cst[:])
            f2 = wk.tile([128, B], f32, name="fin2", tag="fin2")
            nc.gpsimd.memset(f2[:], 0.0)
            nc.vector.tensor_copy(f2[:64, :], hsb_last[2][:64, :])
            nc.sync.dma_start(out=hT2_d[:], in_=f2[:])
            nc.sync.dma_start(out=cT2_d[:], in_=c2[:])

    nc.compile()
    return nc


# ----------------------------------------------------------------------------
# entry point
# ----------------------------------------------------------------------------

_cached = {}


def _get_program(t_steps=T):
    if t_steps not in _cached:
        _cached[t_steps] = build_program(t_steps)
    return _cached[t_steps]


def run(inputs, t_steps=T, trace=False, trace_kwargs=None):
    from concourse.bass_utils import run_bass_kernel_spmd
    nc = _get_program(t_steps)
    in_maps = prepare_inputs(inputs, t_steps)
    res = run_bass_kernel_spmd(nc, in_maps, core_ids=list(range(NCORES)),
                               trace=trace, **(trace_kwargs or {}))
    return res


def assemble_outputs(results, inputs, t_steps=T):
    TB_ = t_steps * B
    decoded = np.empty((t_steps, B, V), np.float32)
    for k in range(NCORES):
        d = results[k]['dec'].reshape(t_steps, B, VS)
        vlo, vhi = VS * k, min(V, VS * (k + 1))
        decoded[:, :, vlo:vhi] = d[:, :, :vhi - vlo]
    dec_b = np.asarray(inputs['dec_b'], np.float32)
    if np.any(dec_b):
        decoded += dec_b

    def unpack_l01(key, hsz):
        out = np.zeros((B, hsz), np.float32)
        for k in range(NCORES):
            a = results[k][key]          # [128, 2, B]
            sl = np.concatenate([a[:, 0, :], a[:32, 1, :]], axis=0)  # [160, B]
            for i, j in enumerate(_slice_rows_l01(k)):
                if j < hsz:
                    out[:, j] = sl[i]
        return out

    def unpack_l2(key, hsz):
        out = np.zeros((B, hsz), np.float32)
        for k in range(NCORES):
            a = results[k][key]          # [128, B]
            lo, hi = SL2 * k, min(hsz, SL2 * (k + 1))
            if hi > lo:
                out[:, lo:hi] = a[:hi - lo].T
        return out

    hT0 = unpack_l01('hT0', H); cT0 = unpack_l01('cT0', H)
    hT1 = unpack_l01('hT1', H); cT1 = unpack_l01('cT1', H)
    hT2 = unpack_l2('hT2', E); cT2 = unpack_l2('cT2', E)
    return (decoded, hT0, cT0, hT1, cT1, hT2, cT2)


def kernel(**inputs):
    res = run(inputs, t_steps=T)
    return assemble_outputs(res.results, inputs, t_steps=T)
